# revision 35
# baseline (speedup 1.0000x reference)
"""Trainium2 Bass kernel for nn_Block_16621523436203 (Mamba-style block).

Sharding: pure data-parallel — batch B=8, one batch element per NeuronCore,
no collectives.  Weights are preprocessed (transposed / LN-folded / cast) on
host; each core runs the full block for its batch element.

Engine plan (per core).  HW constraints found the hard way: GPSIMD (Pool)
cannot touch PSUM and only runs plain TensorTensor (0.42 eff) + DMA;
tensor_tensor_scan is DVE-only; engine outputs consumed by f32r matmuls
must be written with f32r out-dtype; Memset cannot write f32r.

  P1  LN1 -> in_proj -> dwconv+SiLU (x and z) -> x_proj -> dt_proj/softplus
      PE: matmuls+transposes (bf16 weights), ACT: rsqrt(=Sqrt+recip)/SiLU/
      softplus (Exps then Lns batched for table locality), DVE: LN stats,
      evacuations, du mult.  z branch is emitted last so it overlaps the
      scan startup.  Weight DMAs are ordered by first use behind xin on the
      SP HWDGE queue.
  P2  selective scan, 64 groups of (8ch x 16st) partitions, processed in
      pairs ([128, 2048] tiles amortize fixed op cost):
      PE: delta-broadcast matmul (f32r) + D*u diag matmul + n-reduction
      matmul, ACT: exp(dA) + y_cm psum evac, DMA: du broadcast,
      Pool: dBu = du_bc*B mults, DVE: all scans + yt = hs*C mults.
      fc1/fc2 weights stream in via the ACT HWDGE queue during the scan.
  P3  out_proj -> +x -> LN2 -> (transpose -> fc1+GELU -> fc2 + residual)
      pipelined in L-halves; PE-bound.
ACT function-table thrash is minimized (Sqrt/Silu/Exp/Ln/Gelu runs).
"""

import sys

sys.path.insert(0, "/opt/trn_rl_repo")

import os

import ml_dtypes
import numpy as np

import concourse.bacc as bacc
import concourse.bass as bass
import concourse.mybir as mybir
import concourse.tile as tile

F32 = mybir.dt.float32
F32R = mybir.dt.float32r
BF16 = mybir.dt.bfloat16
AF = mybir.ActivationFunctionType
ALU = mybir.AluOpType

B, L, D = 8, 1024, 512
E = 1024  # d_inner
D2 = 512  # per-branch channels
R = 32  # dt_rank
NS = 16  # d_state
KC = 4  # conv kernel size
H = 2048  # mlp hidden
NCORES = 8
TT = L // 128  # 8 token tiles
DC = D // 128  # 4 d_model chunks
D2T = D2 // 128  # 4 channel tiles
ET = E // 128  # 8 d_inner tiles
HT = H // 128  # 16 hidden tiles
NG = 64  # scan groups: each = 8 channels x 16 states
EPS = 1e-5

_BF = ml_dtypes.bfloat16


def _f32r(ap):
    return ap.bitcast(F32R)


STOP_AFTER = int(os.environ.get("KSTOP", "3"))
KREPEAT = int(os.environ.get("KREPEAT", "1"))
KALLOC = int(os.environ.get("KALLOC", "0")) or KREPEAT


def build_kernel(shared_scale=True):
    nc = bacc.Bacc("TRN2", target_bir_lowering=False, debug=False, num_devices=1)

    din = {}

    def inp(name, shape, dtype):
        din[name] = nc.dram_tensor(name, list(shape), dtype, kind="ExternalInput")
        return din[name]

    inp("xin", (KALLOC * L, D), BF16)
    inp("w_inT", (128, DC * E), BF16)  # ln1-folded in_proj weight, d-major blocks
    inp("c_in", (128, ET), F32)  # in_proj bias column per e-tile (W' @ ln1_b)
    inp("diag_x", (128, D2T * KC * 128), BF16)  # conv diag matrices side by side
    inp("diag_z", (128, D2T * KC * 128), BF16)
    inp("x_projT", (128, D2T * (R + 2 * NS)), BF16)
    inp("dt_projT", (R, D2), BF16)
    inp("dt_bias", (128, D2T), F32)
    inp("A_perm", (128, NG), F32)  # A[d(p), n(p)] per group column
    inp("v_col", (128, 1), F32)  # shared exp scale when A is rank-1 (A[c,n]=v[n])
    inp("diag_D", (128, D2T * 128), BF16)  # diag(D) per channel tile
    inp("sel", (128, 16 * 128), BF16)  # n-reduction matmul: SEL[q] blocks
    inp("out_projT", (128, ET * D), BF16)
    inp("fc1T", (128, DC * H), BF16)  # ln2-folded fc1 weight
    inp("c_fc1", (128, HT), F32)  # fc1' @ ln2_b + fc1_b per h-tile
    inp("fc2T", (128, HT * D), BF16)
    inp("fc2b", (1, D), F32R)
    inp("ident_bf", (128, 128), BF16)
    inp("zpad", (128, 3), BF16)
    inp("ones1d", (1, 128), F32R)
    inp("rep_b", (2 * NS, 128), BF16)
    inp("rep_c", (2 * NS, 128), BF16)
    inp("ident_f", (128, 128), F32)

    out_d = nc.dram_tensor("out", [KALLOC * L, D], F32, kind="ExternalOutput")

    with tile.TileContext(nc) as tc:
        for rep_i in range(KREPEAT):
            _body(nc, tc, din, out_d, rep_i * L, shared_scale=shared_scale)
    nc.compile()
    return nc


def _body(nc, tc, din, out_d, row0=0, shared_scale=True):
    xin = din["xin"].ap()[row0 : row0 + L, :]
    out_ap = out_d.ap()[row0 : row0 + L, :]
    HF = L // 2  # half length; the scan is chained across halves so that
    # phase-1 (h1) and phase-3 (h0) matmul work overlaps the DVE-bound scan

    with (
        tc.tile_pool(name="pW3", bufs=1) as pW3,  # weights alive to the end
        tc.tile_pool(name="p13", bufs=1) as p13,  # crosses into phase 3
        tc.tile_pool(name="p12", bufs=1) as p12,  # dies after the scan
        tc.tile_pool(name="scanS", bufs=1) as scanS,  # scan tiles, both windows
        tc.tile_pool(name="psY", bufs=2, space="PSUM") as psY,
    ):
        out_projT = pW3.tile([128, ET * D], BF16, name="opT", tag="opT")
        sel = pW3.tile([128, 16 * 128], BF16, name="sel", tag="sel")
        diag_D = pW3.tile([128, D2T * 128], BF16, name="diagD", tag="diagD")
        fc1T = pW3.tile([128, DC * H], BF16, name="fc1T", tag="fc1T")
        fc2T = pW3.tile([128, HT * D], BF16, name="fc2T", tag="fc2T")
        c_fc1 = pW3.tile([128, HT], F32, name="cfc1", tag="cfc1")
        fc2b = pW3.tile([1, D], F32R, name="fc2b", tag="fc2b")
        ones1 = pW3.tile([1, 128], F32R, name="ones1", tag="ones1")
        ident_bf = pW3.tile([128, 128], BF16, name="idbf", tag="idbf")
        ident_f = pW3.tile([128, 128], F32, name="idf", tag="idf")
        nc.sync.dma_start(out=ident_f[:, :], in_=din["ident_f"].ap()[:, :])
        nc.sync.dma_start(out=ident_bf[:, :], in_=din["ident_bf"].ap()[:, :])

        # phase1->3 tensors
        zh = [p13.tile([128, L], BF16, name=f"zh{i}", tag=f"zh{i}") for i in range(D2T)]
        y_cm = [p13.tile([128, L], BF16, name=f"ycm{i}", tag=f"ycm{i}") for i in range(D2T)]
        x_res = [p13.tile([128, D], BF16, name=f"xres{i}", tag=f"xres{i}") for i in range(TT)]
        # phase1->2 tensors
        xh = [p12.tile([128, L], BF16, name=f"xh{i}", tag=f"xh{i}") for i in range(D2T)]
        # dud[dt] = [du_h0 | dl_h0 | du_h1 | dl_h1] 512-blocks: one bcast DMA
        # per (group, half) carries du and delta together
        dud = [
            p12.tile([128, 2 * L], BF16, name=f"dud{i}", tag=f"dud{i}")
            for i in range(D2T)
        ]
        bbc = p12.tile([128, L], BF16, name="bbc", tag="bbc")
        cbc = p12.tile([128, L], BF16, name="cbc", tag="cbc")
        xdbl_dt = p12.tile([R, L], BF16, name="xdbl", tag="xdbl")
        bc_sb = p12.tile([2 * NS, L], BF16, name="bc_sb", tag="bc_sb")
        a_perm = p12.tile([128, NG], F32, name="aperm", tag="aperm")
        v_col = p12.tile([128, 1], F32, name="vcol", tag="vcol")
        dt_bias = p12.tile([128, D2T], F32, name="dtb", tag="dtb")
        c_in = p12.tile([128, ET], F32, name="cin", tag="cin")
        hfin = scanS.tile([128, NG], F32, name="hfin", tag="hfin")

        nc.sync.dma_start(out=c_in[:, :], in_=din["c_in"].ap()[:, :])
        eps_t = p12.tile([128, 1], F32, name="eps_t", tag="eps_t")
        nc.vector.memset(eps_t[:, :], EPS)

        # ---------- scan block: one (channel-tile, half) = 16 groups ----------
        # fillers: queue of closures emitting ~1-7us of independent work;
        # popped after each pair's sel matmuls so the in-order PE queue has
        # ready work adjacent to the dependency-stalled scan matmuls
        def scan_block(dt, h, fillers=None):
            ps_y = psY.tile([128, HF], F32, name="ps_y", tag="ps_y")
            nc.tensor.matmul(
                ps_y[:, :],
                diag_D[:, dt * 128 : (dt + 1) * 128],
                xh[dt][:, h * HF : (h + 1) * HF],
                start=True,
                stop=False,
            )
            for qp in range(8):
                q0 = 2 * qp
                bdl = scanS.tile([128, 4 * HF], BF16, name="bdl", tag="bdl", bufs=4)
                dA = scanS.tile([128, 2 * HF], F32, name="dA", tag="dA", bufs=3)
                dBu = scanS.tile([128, 2 * HF], BF16, name="dBu", tag="dBu", bufs=4)
                hs = scanS.tile([128, 2 * HF], BF16, name="hs", tag="hs", bufs=3)
                yt = dBu  # dBu dead after the scans; reuse for yt
                for g in range(2):
                    q = q0 + g
                    nc.sync.dma_start(
                        out=bdl[:, g * 2 * HF : (g + 1) * 2 * HF],
                        in_=dud[dt][q * 8 : (q + 1) * 8, h * 2 * HF : (h + 1) * 2 * HF]
                        .unsqueeze(1)
                        .broadcast_to([8, NS, 2 * HF]),
                    )
                bdl4 = bdl[:, :].rearrange("p (b l) -> p b l", b=4)
                if shared_scale:
                    nc.scalar.activation(
                        out=dA[:, :].rearrange("p (b l) -> p b l", b=2),
                        in_=bdl4[:, 1::2, :],
                        func=AF.Exp,
                        bias=0.0,
                        scale=v_col[:, 0:1],
                    )
                else:
                    for g in range(2):
                        G = dt * 16 + q0 + g
                        nc.scalar.activation(
                            out=dA[:, g * HF : (g + 1) * HF],
                            in_=bdl[:, g * 2 * HF + HF : (g + 1) * 2 * HF],
                            func=AF.Exp,
                            bias=0.0,
                            scale=a_perm[:, G : G + 1],
                        )
                dbu_eng = nc.vector if qp % 2 == 0 else nc.gpsimd
                dbu_eng.tensor_tensor(
                    out=dBu[:, :].rearrange("p (b l) -> p b l", b=2),
                    in0=bdl4[:, 0::2, :],
                    in1=bbc[:, h * HF : (h + 1) * HF]
                    .unsqueeze(1)
                    .broadcast_to([128, 2, HF]),
                    op=ALU.mult,
                )
                for g in range(2):
                    G = dt * 16 + q0 + g
                    init = 0.0 if h == 0 else hfin[:, G : G + 1]
                    nc.vector.tensor_tensor_scan(
                        hs[:, g * HF : (g + 1) * HF],
                        dA[:, g * HF : (g + 1) * HF],
                        dBu[:, g * HF : (g + 1) * HF],
                        init,
                        ALU.mult,
                        ALU.add,
                    )
                if h == 0:
                    nc.vector.tensor_copy(
                        hfin[:, dt * 16 + q0 : dt * 16 + q0 + 2],
                        hs[:, :].rearrange("p (g l) -> p g l", g=2)[:, :, HF - 1],
                    )
                yt_eng = nc.vector if qp % 2 == 1 else nc.gpsimd
                yt_eng.tensor_tensor(
                    out=yt[:, :],
                    in0=hs[:, :],
                    in1=cbc[:, h * HF : (h + 1) * HF]
                    .unsqueeze(1)
                    .broadcast_to([128, 2, HF]),
                    op=ALU.mult,
                )
                for g in range(2):
                    q = q0 + g
                    nc.tensor.matmul(
                        ps_y[:, :],
                        sel[:, q * 128 : (q + 1) * 128],
                        yt[:, g * HF : (g + 1) * HF],
                        start=False,
                        stop=(q == 15),
                    )
                if fillers:
                    fillers.popleft()()
            nc.scalar.copy(out=y_cm[dt][:, h * HF : (h + 1) * HF], in_=ps_y[:, :])

        # ================= P1a + window 1 ==========
        with (
            tc.tile_pool(name="wE", bufs=1) as wE,
            tc.tile_pool(name="xpP", bufs=1) as xpP,
            tc.tile_pool(name="t1", bufs=2) as t1,
            tc.tile_pool(name="ts", bufs=3) as ts,
            tc.tile_pool(name="tsp", bufs=1) as tsp,
            tc.tile_pool(name="xhatT_p", bufs=1) as xhatT_p,
            tc.tile_pool(name="psTr", bufs=1, space="PSUM") as psTr,
            tc.tile_pool(name="psIn", bufs=2, space="PSUM") as psIn,
            tc.tile_pool(name="psTiny", bufs=1, space="PSUM") as psTiny,
            tc.tile_pool(name="psConv", bufs=2, space="PSUM") as psConv,
        ):
            w_inT = wE.tile([128, DC * E], BF16, name="winT", tag="winT")
            diag = {}
            for br in ("x", "z"):
                diag[br] = wE.tile(
                    [128, D2T * KC * 128], BF16, name=f"diag{br}", tag=f"diag{br}"
                )
            x_projT = wE.tile([128, D2T * (R + 2 * NS)], BF16, name="xpj", tag="xpj")
            dt_projT = wE.tile([R, D2], BF16, name="dtpj", tag="dtpj")
            rep_b = wE.tile([2 * NS, 128], BF16, name="rep_b", tag="rep_b")
            rep_c = wE.tile([2 * NS, 128], BF16, name="rep_c", tag="rep_c")

            xhatT = [
                xhatT_p.tile([128, L], BF16, name=f"xhT{i}", tag=f"xhT{i}")
                for i in range(DC)
            ]

            # ---- LN1 (token-major) + transpose, all 8 token tiles ----
            # PE clock warmup: burn idle DMA-wait time on dummy matmuls so
            # in_proj runs at full clock.
            ps_w = psTiny.tile([128, 128], F32, name="ps_w", tag="ps_w")
            for _ in range(8):
                nc.tensor.matmul(
                    ps_w[:, :], ident_f[:, :], ident_f[:, :], start=True, stop=True
                )
            for tt in range(TT):
                x_t = x_res[tt]
                nc.sync.dma_start(out=x_t[:, :], in_=xin[tt * 128 : (tt + 1) * 128, :])
                stats = ts.tile([128, 6], F32, name="stats", tag="stats")
                nc.vector.bn_stats(out=stats[:, :], in_=x_t[:, :])
                mv = ts.tile([128, 2], F32, name="mv", tag="mv")
                nc.vector.bn_aggr(out=mv[:, :], in_=stats[:, :])
                sd = ts.tile([128, 1], F32, name="sd", tag="sd")
                nc.scalar.activation(
                    out=sd[:, :], in_=mv[:, 1:2], func=AF.Sqrt, bias=eps_t[:, :], scale=1.0
                )
                r_t = ts.tile([128, 1], F32, name="r_t", tag="r_t")
                nc.vector.reciprocal(out=r_t[:, :], in_=sd[:, :])
                xhat = t1.tile([128, D], BF16, name="xhat", tag="xhat")
                nc.vector.tensor_scalar(
                    out=xhat[:, :],
                    in0=x_t[:, :],
                    scalar1=mv[:, 0:1],
                    scalar2=r_t[:, :],
                    op0=ALU.subtract,
                    op1=ALU.mult,
                )
                # keep PE continuously busy between transposes (pstate)
                for _ in range(3):
                    nc.tensor.matmul(
                        ps_w[:, :], ident_f[:, :], ident_f[:, :], start=True, stop=True
                    )
                ps_tr = psTr.tile([128, D], BF16, name="ps_tr", tag="ps_tr")
                for dc in range(DC):
                    nc.tensor.transpose(
                        ps_tr[:, dc * 128 : (dc + 1) * 128],
                        xhat[:, dc * 128 : (dc + 1) * 128],
                        ident_bf[:, :],
                    )
                for dc in range(DC):
                    nc.vector.tensor_copy(
                        xhatT[dc][:, tt * 128 : (tt + 1) * 128],
                        ps_tr[:, dc * 128 : (dc + 1) * 128],
                    )

            # weight DMAs behind xin on the SP FIFO queue, ordered by first use
            nc.sync.dma_start(out=w_inT[:, :], in_=din["w_inT"].ap()[:, :])
            nc.sync.dma_start(out=diag["x"][:, :], in_=din["diag_x"].ap()[:, :])
            nc.sync.dma_start(out=x_projT[:, :], in_=din["x_projT"].ap()[:, :])
            nc.sync.dma_start(out=dt_projT[:, :], in_=din["dt_projT"].ap()[:, :])
            nc.sync.dma_start(out=rep_b[:, :], in_=din["rep_b"].ap()[:, :])
            nc.sync.dma_start(out=rep_c[:, :], in_=din["rep_c"].ap()[:, :])
            nc.sync.dma_start(out=dt_bias[:, :], in_=din["dt_bias"].ap()[:, :])
            nc.sync.dma_start(out=sel[:, :], in_=din["sel"].ap()[:, :])
            nc.sync.dma_start(out=a_perm[:, :], in_=din["A_perm"].ap()[:, :])
            nc.sync.dma_start(out=v_col[:, :], in_=din["v_col"].ap()[:, :])
            nc.sync.dma_start(out=diag_D[:, :], in_=din["diag_D"].ap()[:, :])
            nc.sync.dma_start(out=diag["z"][:, :], in_=din["diag_z"].ap()[:, :])
            nc.sync.dma_start(out=out_projT[:, :], in_=din["out_projT"].ap()[:, :])
            nc.sync.dma_start(out=c_fc1[:, :], in_=din["c_fc1"].ap()[:, :])
            nc.sync.dma_start(out=fc2b[:, :], in_=din["fc2b"].ap()[:, :])
            nc.sync.dma_start(out=ones1[:, :], in_=din["ones1d"].ap()[:, :])

            # ---- conv input buffers (padded by 1 left / 2 right) ----
            xp = {
                "x": [
                    xpP.tile([128, L + 3], BF16, name=f"xpx{i}", tag=f"xpx{i}")
                    for i in range(D2T)
                ],
                "z": [
                    xpP.tile([128, L + 3], BF16, name=f"xpz{i}", tag=f"xpz{i}")
                    for i in range(D2T)
                ],
            }
            for br in ("x", "z"):
                for dtc in range(D2T):
                    nc.sync.dma_start(out=xp[br][dtc][:, 0:1], in_=din["zpad"].ap()[:, 0:1])
                    nc.sync.dma_start(
                        out=xp[br][dtc][:, L + 1 : L + 3], in_=din["zpad"].ap()[:, 0:2]
                    )

            # in_proj token ranges: h0 covers [0,514) (conv lookahead), h1 the rest
            def in_proj_half(et, h):
                br, dtc = ("x", et) if et < D2T else ("z", et - D2T)
                chunks = [(0, 512), (512, 514)] if h == 0 else [(514, 1024)]
                for c0, c1 in chunks:
                    w = c1 - c0
                    if w > 16:
                        ps = psIn.tile([128, 512], F32, name="ps_inp", tag="ps_inp")
                    else:
                        ps = psTiny.tile([128, 128], F32, name="ps_w", tag="ps_w")
                    for dc in range(DC):
                        nc.tensor.matmul(
                            ps[:, 0:w],
                            w_inT[:, dc * E + et * 128 : dc * E + (et + 1) * 128],
                            xhatT[dc][:, c0:c1],
                            start=(dc == 0),
                            stop=(dc == DC - 1),
                        )
                    nc.vector.tensor_scalar(
                        out=xp[br][dtc][:, 1 + c0 : 1 + c1],
                        in0=ps[:, 0:w],
                        scalar1=c_in[:, et : et + 1],
                        scalar2=None,
                        op0=ALU.add,
                    )

            def conv_half(br, dtc, h, raw=False):
                ps = psConv.tile([128, 512], F32, name="ps_conv", tag="ps_conv")
                for j in range(KC):
                    nc.tensor.matmul(
                        ps[:, :],
                        diag[br][:, (dtc * KC + j) * 128 : (dtc * KC + j + 1) * 128],
                        xp[br][dtc][:, h * HF + j : h * HF + j + HF],
                        start=(j == 0),
                        stop=(j == KC - 1),
                    )
                dst = xh[dtc] if br == "x" else zh[dtc]
                if raw:
                    # table-free ACT Copy evac (ready straight from the PE
                    # psum, so the following in-place silu island coheres);
                    # silu applied in place later to avoid Exp<->Silu thrash
                    nc.scalar.copy(out=dst[:, h * HF : (h + 1) * HF], in_=ps[:, :])
                else:
                    nc.scalar.activation(
                        out=dst[:, h * HF : (h + 1) * HF],
                        in_=ps[:, :],
                        func=AF.Silu,
                        bias=0.0,
                        scale=1.0,
                    )

            def silu_island(specs):
                for br, dtc, h in specs:
                    dst = xh[dtc] if br == "x" else zh[dtc]
                    nc.scalar.activation(
                        out=dst[:, h * HF : (h + 1) * HF],
                        in_=dst[:, h * HF : (h + 1) * HF],
                        func=AF.Silu,
                        bias=0.0,
                        scale=1.0,
                    )

            def xproj_half(h):
                RW = R + 2 * NS
                ps = psIn.tile([128, 512], F32, name="ps_xd", tag="ps_inp")
                for dtc in range(D2T):
                    nc.tensor.matmul(
                        ps[0:RW, :],
                        x_projT[:, dtc * RW : (dtc + 1) * RW],
                        xh[dtc][:, h * HF : (h + 1) * HF],
                        start=(dtc == 0),
                        stop=(dtc == D2T - 1),
                    )
                nc.vector.tensor_copy(xdbl_dt[:, h * HF : (h + 1) * HF], ps[0:R, :])
                nc.vector.tensor_copy(
                    bc_sb[:, h * HF : (h + 1) * HF], ps[R : R + 2 * NS, :]
                )
                for dst_t, rep_t in ((bbc, rep_b), (cbc, rep_c)):
                    ps2 = psIn.tile([128, 512], F32, name="ps_bc", tag="ps_inp")
                    nc.tensor.matmul(
                        ps2[:, :],
                        rep_t[:, :],
                        bc_sb[:, h * HF : (h + 1) * HF],
                        start=True,
                        stop=True,
                    )
                    nc.vector.tensor_copy(dst_t[:, h * HF : (h + 1) * HF], ps2[:, :])

            def dt_soft_half(h, du_eng):
                # Exps batched before Lns (same ACT table set covers both)
                t_sps = []
                for dtc in range(D2T):
                    ps3 = psConv.tile([128, 512], F32, name="ps_dt", tag="ps_conv")
                    nc.tensor.matmul(
                        ps3[:, :],
                        dt_projT[:, dtc * 128 : (dtc + 1) * 128],
                        xdbl_dt[:, h * HF : (h + 1) * HF],
                        start=True,
                        stop=True,
                    )
                    t_sp = tsp.tile(
                        [128, 512], F32, name=f"tsp{dtc}", tag=f"tsp{dtc}", bufs=1
                    )
                    nc.scalar.activation(
                        out=t_sp[:, :],
                        in_=ps3[:, :],
                        func=AF.Exp,
                        bias=dt_bias[:, dtc : dtc + 1],
                        scale=1.0,
                    )
                    t_sps.append(t_sp)
                for dtc in range(D2T):
                    nc.scalar.activation(
                        out=dud[dtc][:, h * 2 * HF + HF : (h + 1) * 2 * HF],
                        in_=t_sps[dtc][:, :],
                        func=AF.Ln,
                        bias=1.0,
                        scale=1.0,
                    )
                    du_eng.tensor_tensor(
                        out=dud[dtc][:, h * 2 * HF : h * 2 * HF + HF],
                        in0=dud[dtc][:, h * 2 * HF + HF : (h + 1) * 2 * HF],
                        in1=xh[dtc][:, h * HF : (h + 1) * HF],
                        op=ALU.mult,
                    )

            # ---- P1a: everything the h0 scan needs ----
            for et in range(D2T):
                in_proj_half(et, 0)
            for dtc in range(D2T):
                conv_half("x", dtc, 0)
            xproj_half(0)
            dt_soft_half(0, nc.gpsimd)

            # ---- window 1: h0 scan blocks with P1b work in the pair slots ----
            from collections import deque

            # silu-bearing convs are merged into single items so their ACT
            # table loads happen once per island, not once per scan pair
            f1 = deque()
            for et in range(D2T):
                f1.append(lambda et=et: in_proj_half(et, 1))
            f1.append(lambda: [conv_half("x", d, 1, raw=True) for d in range(D2T)])
            f1.append(lambda: silu_island([("x", d, 1) for d in range(D2T)]))
            f1.append(lambda: xproj_half(1))
            f1.append(lambda: dt_soft_half(1, nc.gpsimd))
            for et in range(D2T, ET):
                f1.append(
                    lambda et=et: (in_proj_half(et, 0), in_proj_half(et, 1))
                )
            f1.append(
                lambda: [
                    conv_half("z", d, hh, raw=True)
                    for d in range(D2T)
                    for hh in range(2)
                ]
            )
            f1.append(
                lambda: silu_island(
                    [("z", d, hh) for d in range(D2T) for hh in range(2)]
                )
            )
            w1c = DC * H // 4
            for dt in range(D2T):
                scan_block(dt, 0, f1)
                nc.scalar.dma_start(
                    out=fc1T[:, dt * w1c : (dt + 1) * w1c],
                    in_=din["fc1T"].ap()[:, dt * w1c : (dt + 1) * w1c],
                )
            while f1:
                f1.popleft()()

        # ================= window 2 + phase 3 ==========
        with (
            tc.tile_pool(name="p3", bufs=1) as p3,
            tc.tile_pool(name="t3", bufs=2) as t3,
            tc.tile_pool(name="psG3", bufs=2, space="PSUM") as psG3,
            tc.tile_pool(name="psF1", bufs=2, space="PSUM") as psF1,
            tc.tile_pool(name="psTr3", bufs=1, space="PSUM") as psTr3,
        ):
            h_res = [
                p3.tile([128, D], F32, name=f"hres{i}", tag=f"hres{i}") for i in range(TT)
            ]
            xhat2 = [
                p3.tile([128, D], BF16, name=f"xh2{i}", tag=f"xh2{i}") for i in range(TT)
            ]
            xhat2T = [
                p3.tile([128, L], BF16, name=f"xh2T{i}", tag=f"xh2T{i}")
                for i in range(DC)
            ]
            # aT holds one L-half at a time: h0 is consumed by fc2(tt 0..3)
            # before fc1_half(1) overwrites it
            aT = [
                p3.tile([128, 512], BF16, name=f"aT{i}", tag=f"aT{i}")
                for i in range(HT)
            ]

            def out_proj_tt(tt):
                ps = psG3.tile([128, D], F32, name="ps_op", tag="g3")
                korder = list(range(D2T, ET)) + list(range(D2T))
                for ki, k in enumerate(korder):
                    lhs = (
                        y_cm[k][:, tt * 128 : (tt + 1) * 128]
                        if k < D2T
                        else zh[k - D2T][:, tt * 128 : (tt + 1) * 128]
                    )
                    nc.tensor.matmul(
                        ps[:, :],
                        lhs,
                        out_projT[:, k * D : (k + 1) * D],
                        start=(ki == 0),
                        stop=(ki == ET - 1),
                    )
                # + residual on DVE (also evacuates the psum)
                nc.vector.tensor_tensor(
                    out=h_res[tt][:, :], in0=ps[:, :], in1=x_res[tt][:, :], op=ALU.add
                )

            def ln2_stats_tt(tt):
                # ACT-side stats: Square+accum and Identity+accum are in every
                # ACT table set, so they don't thrash the Exp table mid-scan
                sq = t3.tile([128, D], F32, name="sq3", tag="sq3")
                qsum = t3.tile([128, 1], F32, name="qsum", tag="qsum", bufs=8)
                ssum = t3.tile([128, 1], F32, name="ssum", tag="ssum", bufs=8)
                nc.scalar.activation(
                    out=sq[:, :], in_=h_res[tt][:, :], func=AF.Square, accum_out=qsum[:, :]
                )
                nc.scalar.activation(
                    out=sq[:, :], in_=h_res[tt][:, :], func=AF.Identity,
                    accum_out=ssum[:, :],
                )
                return qsum, ssum

            def ln2_finish_batch(tts):
                parts = {}
                for tt in tts:
                    qsum, ssum = ln2_acc[tt]
                    m = t3.tile([128, 1], F32, name="m3", tag="m3", bufs=8)
                    nc.vector.tensor_scalar(
                        out=m[:, :], in0=ssum[:, :], scalar1=1.0 / D, scalar2=None,
                        op0=ALU.mult,
                    )
                    msq = t3.tile([128, 1], F32, name="msq", tag="msq", bufs=8)
                    nc.vector.tensor_tensor(
                        out=msq[:, :], in0=m[:, :], in1=m[:, :], op=ALU.mult
                    )
                    v = t3.tile([128, 1], F32, name="v3", tag="v3", bufs=8)
                    nc.vector.scalar_tensor_tensor(
                        out=v[:, :], in0=qsum[:, :], scalar=1.0 / D, in1=msq[:, :],
                        op0=ALU.mult, op1=ALU.subtract,
                    )
                    parts[tt] = (m, v)
                sds = {}
                for tt in tts:
                    m, v = parts[tt]
                    sd = t3.tile([128, 1], F32, name="sd3", tag="sd3", bufs=8)
                    nc.scalar.activation(
                        out=sd[:, :], in_=v[:, :], func=AF.Sqrt, bias=eps_t[:, :],
                        scale=1.0,
                    )
                    sds[tt] = sd
                for tt in tts:
                    m, v = parts[tt]
                    r_t = t3.tile([128, 1], F32, name="r3", tag="r3", bufs=8)
                    nc.vector.reciprocal(out=r_t[:, :], in_=sds[tt][:, :])
                    nc.vector.tensor_scalar(
                        out=xhat2[tt][:, :],
                        in0=h_res[tt][:, :],
                        scalar1=m[:, 0:1],
                        scalar2=r_t[:, :],
                        op0=ALU.subtract,
                        op1=ALU.mult,
                    )

            def ln2_finish_tt(tt, qsum, ssum):
                # m = ssum/512; v = qsum/512 - m^2; r = 1/sqrt(v+eps); all tiny
                m = t3.tile([128, 1], F32, name="m3", tag="m3")
                nc.vector.tensor_scalar(
                    out=m[:, :], in0=ssum[:, :], scalar1=1.0 / D, scalar2=None,
                    op0=ALU.mult,
                )
                # v = qsum/512 - m*m
                msq = t3.tile([128, 1], F32, name="msq", tag="msq")
                nc.vector.tensor_tensor(
                    out=msq[:, :], in0=m[:, :], in1=m[:, :], op=ALU.mult
                )
                v = t3.tile([128, 1], F32, name="v3", tag="v3")
                nc.vector.scalar_tensor_tensor(
                    out=v[:, :], in0=qsum[:, :], scalar=1.0 / D, in1=msq[:, :],
                    op0=ALU.mult, op1=ALU.subtract,
                )
                sd = t3.tile([128, 1], F32, name="sd3", tag="sd3")
                nc.scalar.activation(
                    out=sd[:, :], in_=v[:, :], func=AF.Sqrt, bias=eps_t[:, :], scale=1.0
                )
                r_t = t3.tile([128, 1], F32, name="r3", tag="r3")
                nc.vector.reciprocal(out=r_t[:, :], in_=sd[:, :])
                nc.vector.tensor_scalar(
                    out=xhat2[tt][:, :],
                    in0=h_res[tt][:, :],
                    scalar1=m[:, 0:1],
                    scalar2=r_t[:, :],
                    op0=ALU.subtract,
                    op1=ALU.mult,
                )

            def tr3_dc(half, dc):
                ps_t = psTr3.tile([128, 512], BF16, name="ps_t3", tag="ps_t3")
                for b4 in range(4):
                    tt = half * 4 + b4
                    nc.tensor.transpose(
                        ps_t[:, b4 * 128 : (b4 + 1) * 128],
                        xhat2[tt][:, dc * 128 : (dc + 1) * 128],
                        ident_bf[:, :],
                    )
                nc.scalar.copy(
                    out=xhat2T[dc][:, half * 512 : (half + 1) * 512], in_=ps_t[:, :]
                )

            def fc1_block(half, ht0, ht1, raw=False):
                for ht in range(ht0, ht1):
                    ps = psF1.tile([128, 512], F32, name="ps_fc1", tag="f1")
                    for dc in range(DC):
                        nc.tensor.matmul(
                            ps[:, :],
                            fc1T[:, dc * H + ht * 128 : dc * H + (ht + 1) * 128],
                            xhat2T[dc][:, half * 512 : (half + 1) * 512],
                            start=(dc == 0),
                            stop=(dc == DC - 1),
                        )
                    if raw:
                        nc.scalar.copy(out=aT[ht][:, :], in_=ps[:, :])
                    else:
                        nc.scalar.activation(
                            out=aT[ht][:, :],
                            in_=ps[:, :],
                            func=AF.Gelu,
                            bias=c_fc1[:, ht : ht + 1],
                            scale=1.0,
                        )

            def gelu_island():
                for ht in range(HT):
                    nc.scalar.activation(
                        out=aT[ht][:, :],
                        in_=aT[ht][:, :],
                        func=AF.Gelu,
                        bias=c_fc1[:, ht : ht + 1],
                        scale=1.0,
                    )

            def fc2_tt(tt):
                ps = psG3.tile([128, D], F32, name="ps_fc2", tag="g3")
                for ht in range(HT):
                    nc.tensor.matmul(
                        ps[:, :],
                        aT[ht][:, (tt % 4) * 128 : (tt % 4 + 1) * 128],
                        fc2T[:, ht * D : (ht + 1) * D],
                        start=(ht == 0),
                        stop=False,
                    )
                nc.tensor.matmul(
                    ps[:, :], ones1[:, :], fc2b[:, :], start=False, stop=True
                )
                o_t = t3.tile([128, D], F32, name="o_t", tag="o_t")
                nc.vector.tensor_tensor(
                    out=o_t[:, :], in0=ps[:, :], in1=h_res[tt][:, :], op=ALU.add
                )
                nc.sync.dma_start(
                    out=out_ap[tt * 128 : (tt + 1) * 128, :], in_=o_t[:, :]
                )

            # ---- window 2: h1 scan blocks with phase-3 h0 in the pair slots ----
            from collections import deque

            ln2_acc = {}

            def op_item(tt):
                out_proj_tt(tt)
                ln2_acc[tt] = ln2_stats_tt(tt)

            def ln2_batch(tts):
                ln2_finish_batch(list(tts))

            f2 = deque()
            for tt in range(4):
                f2.append(lambda t=tt: op_item(t))
            f2.append(lambda: ln2_batch(range(4)))
            for dc in range(DC):
                f2.append(lambda d=dc: tr3_dc(0, d))
            # fc1 evacs are raw copies; one in-place gelu island bounds the
            # ACT table switches to a single Exp<->Gelu round trip
            f2.append(lambda: fc1_block(0, 0, 8, raw=True))
            f2.append(lambda: fc1_block(0, 8, 16, raw=True))
            f2.append(gelu_island)
            for tt in range(4):
                f2.append(lambda t=tt: fc2_tt(t))

            # fc2T must be emitted before the fc2 filler items (tile deps
            # bind at emission order); stream in two chunks up front
            w2c = HT * D // 2
            for ck in range(2):
                nc.scalar.dma_start(
                    out=fc2T[:, ck * w2c : (ck + 1) * w2c],
                    in_=din["fc2T"].ap()[:, ck * w2c : (ck + 1) * w2c],
                )
            for dt in range(D2T):
                scan_block(dt, 1, f2)

            # ---- tail: phase-3 for h1 ----
            while f2:
                f2.popleft()()
            for tt in range(4, TT):
                op_item(tt)
            ln2_batch(range(4, TT))
            for dc in range(DC):
                tr3_dc(1, dc)
            fc1_block(1, 0, 16)
            for tt in range(4, TT):
                fc2_tt(tt)


def _mk_repbc(row0):
    m = np.zeros((2 * NS, 128), np.float32)
    p = np.arange(128)
    m[row0 + (p % 16), p] = 1.0
    return m


def prep_inputs(inputs):
    """Host-side weight preprocessing. Returns the shared (non-x) in_map."""
    g = {k: np.asarray(v, dtype=np.float32) for k, v in inputs.items()}

    ln1_w, ln1_b = g["ln1_w"], g["ln1_b"]
    ln2_w, ln2_b = g["ln2_w"], g["ln2_b"]

    w_in = g["in_proj_w"] * ln1_w[None, :]  # [E, D]
    c_in = (g["in_proj_w"] @ ln1_b).astype(np.float32)  # [E]

    fc1 = g["fc1_w"] * ln2_w[None, :]  # [H, D]
    c_fc1 = (g["fc1_w"] @ ln2_b + g["fc1_b"]).astype(np.float32)  # [H]

    A = -np.exp(g["A_log"])  # [D2, NS]

    # REP[q][k, m] = 1 iff k == q*8 + m//16   (delta row broadcast)
    rep = np.zeros((16, 128, 128), np.float32)
    for q in range(16):
        m = np.arange(128)
        rep[q, q * 8 + m // 16, m] = 1.0
    # SEL[q][k, m] = 1 iff m == q*8 + k//16   (sum over n into channel rows)
    sel = np.transpose(rep, (0, 2, 1)).copy()
    # A_perm[p, g] = A[g*8 + p//16, p%16]
    p = np.arange(128)
    gg = np.arange(NG)
    A_perm = A[(gg[None, :] * 8 + (p // 16)[:, None]), (p % 16)[:, None]].astype(
        np.float32
    )
    # shared-scale fast path: A rows identical across channels (A[c,n] = v[n])
    v = A[0, :]
    a_shared = bool(np.allclose(A, v[None, :], rtol=1e-5, atol=1e-7))
    v_col = v[(p % 16)].reshape(128, 1).astype(np.float32)
    diag_D = np.zeros((D2T, 128, 128), np.float32)
    idx128 = np.arange(128)
    for dt in range(D2T):
        diag_D[dt, idx128, idx128] = g["ssm_D"][dt * 128 : (dt + 1) * 128]

    conv_x = g["conv_x_w"][:, 0, :]  # [D2, KC]
    conv_z = g["conv_z_w"][:, 0, :]
    diag_x = np.zeros((D2T * KC, 128, 128), np.float32)
    diag_z = np.zeros((D2T * KC, 128, 128), np.float32)
    idx = np.arange(128)
    for dt in range(D2T):
        for j in range(KC):
            diag_x[dt * KC + j, idx, idx] = conv_x[dt * 128 : (dt + 1) * 128, j]
            diag_z[dt * KC + j, idx, idx] = conv_z[dt * 128 : (dt + 1) * 128, j]

    def bf(x):
        return np.ascontiguousarray(x.astype(_BF))

    def f(x):
        return np.ascontiguousarray(x.astype(np.float32))

    def blocks(arr3):  # [N, 128, W] -> [128, N*W]
        n, pdim, w = arr3.shape
        assert pdim == 128
        return np.ascontiguousarray(arr3.transpose(1, 0, 2).reshape(128, n * w))

    RW = R + 2 * NS
    shared = {
        "w_inT": bf(blocks(w_in.T.reshape(DC, 128, E))),
        "c_in": f(c_in.reshape(ET, 128).T),
        "diag_x": bf(blocks(diag_x)),
        "diag_z": bf(blocks(diag_z)),
        "x_projT": bf(blocks(g["x_proj_w"].T.reshape(D2T, 128, RW))),
        "dt_projT": bf(g["dt_proj_w"].T),
        "dt_bias": f(g["dt_proj_b"].reshape(D2T, 128).T),
        "A_perm": f(A_perm),
        "v_col": f(v_col),
        "diag_D": bf(blocks(diag_D)),
        "sel": bf(blocks(sel)),
        "out_projT": bf(blocks(g["out_proj_w"].T.reshape(ET, 128, D))),
        "fc1T": bf(blocks(fc1.T.reshape(DC, 128, H))),
        "c_fc1": f(c_fc1.reshape(HT, 128).T),
        "fc2T": bf(blocks(g["fc2_w"].T.reshape(HT, 128, D))),
        "fc2b": f(g["fc2_b"].reshape(1, D)),
        "ident_bf": bf(np.eye(128, dtype=np.float32)),
        "zpad": np.zeros((128, 3), _BF),
        "ones1d": np.ones((1, 128), np.float32),
        "rep_b": bf(_mk_repbc(0)),
        "rep_c": bf(_mk_repbc(NS)),
        "ident_f": np.eye(128, dtype=np.float32),
    }
    return shared, a_shared


_CACHED_NC = {}


def kernel(**inputs):
    from concourse.bass_utils import run_bass_kernel_spmd

    shared, a_shared = prep_inputs(inputs)
    if a_shared not in _CACHED_NC:
        _CACHED_NC[a_shared] = build_kernel(shared_scale=a_shared)
    nc = _CACHED_NC[a_shared]
    x = np.asarray(inputs["x"], dtype=np.float32)
    in_maps = [
        dict(
            shared,
            xin=np.ascontiguousarray(
                np.concatenate([x[i]] * KREPEAT, axis=0).astype(_BF)
            ),
        )
        for i in range(NCORES)
    ]
    res = run_bass_kernel_spmd(nc, in_maps, core_ids=list(range(NCORES)))
    out = np.stack([res.results[i]["out"][:L] for i in range(NCORES)], axis=0)
    return out


if __name__ == "__main__":
    nc = build_kernel()
    print("build ok")



# revision 36
# speedup vs baseline: 1.0004x; 1.0004x over previous
"""Trainium2 Bass kernel for nn_Block_16621523436203 (Mamba-style block).

Sharding: pure data-parallel — batch B=8, one batch element per NeuronCore,
no collectives.  Weights are preprocessed (transposed / LN-folded / cast) on
host; each core runs the full block for its batch element.

Engine plan (per core).  HW constraints found the hard way: GPSIMD (Pool)
cannot touch PSUM and only runs plain TensorTensor (0.42 eff) + DMA;
tensor_tensor_scan is DVE-only; engine outputs consumed by f32r matmuls
must be written with f32r out-dtype; Memset cannot write f32r.

  P1  LN1 -> in_proj -> dwconv+SiLU (x and z) -> x_proj -> dt_proj/softplus
      PE: matmuls+transposes (bf16 weights), ACT: rsqrt(=Sqrt+recip)/SiLU/
      softplus (Exps then Lns batched for table locality), DVE: LN stats,
      evacuations, du mult.  z branch is emitted last so it overlaps the
      scan startup.  Weight DMAs are ordered by first use behind xin on the
      SP HWDGE queue.
  P2  selective scan, 64 groups of (8ch x 16st) partitions, processed in
      pairs ([128, 2048] tiles amortize fixed op cost):
      PE: delta-broadcast matmul (f32r) + D*u diag matmul + n-reduction
      matmul, ACT: exp(dA) + y_cm psum evac, DMA: du broadcast,
      Pool: dBu = du_bc*B mults, DVE: all scans + yt = hs*C mults.
      fc1/fc2 weights stream in via the ACT HWDGE queue during the scan.
  P3  out_proj -> +x -> LN2 -> (transpose -> fc1+GELU -> fc2 + residual)
      pipelined in L-halves; PE-bound.
ACT function-table thrash is minimized (Sqrt/Silu/Exp/Ln/Gelu runs).
"""

import sys

sys.path.insert(0, "/opt/trn_rl_repo")

import os

import ml_dtypes
import numpy as np

import concourse.bacc as bacc
import concourse.bass as bass
import concourse.mybir as mybir
import concourse.tile as tile

F32 = mybir.dt.float32
F32R = mybir.dt.float32r
BF16 = mybir.dt.bfloat16
AF = mybir.ActivationFunctionType
ALU = mybir.AluOpType

B, L, D = 8, 1024, 512
E = 1024  # d_inner
D2 = 512  # per-branch channels
R = 32  # dt_rank
NS = 16  # d_state
KC = 4  # conv kernel size
H = 2048  # mlp hidden
NCORES = 8
TT = L // 128  # 8 token tiles
DC = D // 128  # 4 d_model chunks
D2T = D2 // 128  # 4 channel tiles
ET = E // 128  # 8 d_inner tiles
HT = H // 128  # 16 hidden tiles
NG = 64  # scan groups: each = 8 channels x 16 states
EPS = 1e-5

_BF = ml_dtypes.bfloat16


def _f32r(ap):
    return ap.bitcast(F32R)


STOP_AFTER = int(os.environ.get("KSTOP", "3"))
KREPEAT = int(os.environ.get("KREPEAT", "1"))
KALLOC = int(os.environ.get("KALLOC", "0")) or KREPEAT


def build_kernel(shared_scale=True):
    nc = bacc.Bacc("TRN2", target_bir_lowering=False, debug=False, num_devices=1)

    din = {}

    def inp(name, shape, dtype):
        din[name] = nc.dram_tensor(name, list(shape), dtype, kind="ExternalInput")
        return din[name]

    inp("xin", (KALLOC * L, D), BF16)
    inp("w_inT", (128, DC * E), BF16)  # ln1-folded in_proj weight, d-major blocks
    inp("c_in", (128, ET), F32)  # in_proj bias column per e-tile (W' @ ln1_b)
    inp("diag_x", (128, D2T * KC * 128), BF16)  # conv diag matrices side by side
    inp("diag_z", (128, D2T * KC * 128), BF16)
    inp("x_projT", (128, D2T * (R + 2 * NS)), BF16)
    inp("dt_projT", (R, D2), BF16)
    inp("dt_bias", (128, D2T), F32)
    inp("A_perm", (128, NG), F32)  # A[d(p), n(p)] per group column
    inp("v_col", (128, 1), F32)  # shared exp scale when A is rank-1 (A[c,n]=v[n])
    inp("diag_D", (128, D2T * 128), BF16)  # diag(D) per channel tile
    inp("sel", (128, 16 * 128), BF16)  # n-reduction matmul: SEL[q] blocks
    inp("out_projT", (128, ET * D), BF16)
    inp("fc1T", (128, DC * H), BF16)  # ln2-folded fc1 weight
    inp("c_fc1", (128, HT), F32)  # fc1' @ ln2_b + fc1_b per h-tile
    inp("fc2T", (128, HT * D), BF16)
    inp("fc2b", (1, D), F32R)
    inp("ident_bf", (128, 128), BF16)
    inp("zpad", (128, 3), BF16)
    inp("ones1d", (1, 128), F32R)
    inp("rep_b", (2 * NS, 128), BF16)
    inp("rep_c", (2 * NS, 128), BF16)
    inp("ident_f", (128, 128), F32)

    out_d = nc.dram_tensor("out", [KALLOC * L, D], F32, kind="ExternalOutput")

    with tile.TileContext(nc) as tc:
        for rep_i in range(KREPEAT):
            _body(nc, tc, din, out_d, rep_i * L, shared_scale=shared_scale)
    nc.compile()
    return nc


def _body(nc, tc, din, out_d, row0=0, shared_scale=True):
    xin = din["xin"].ap()[row0 : row0 + L, :]
    out_ap = out_d.ap()[row0 : row0 + L, :]
    HF = L // 2  # half length; the scan is chained across halves so that
    # phase-1 (h1) and phase-3 (h0) matmul work overlaps the DVE-bound scan

    with (
        tc.tile_pool(name="pW3", bufs=1) as pW3,  # weights alive to the end
        tc.tile_pool(name="p13", bufs=1) as p13,  # crosses into phase 3
        tc.tile_pool(name="p12", bufs=1) as p12,  # dies after the scan
        tc.tile_pool(name="scanS", bufs=1) as scanS,  # scan tiles, both windows
        tc.tile_pool(name="psY", bufs=2, space="PSUM") as psY,
    ):
        out_projT = pW3.tile([128, ET * D], BF16, name="opT", tag="opT")
        sel = pW3.tile([128, 16 * 128], BF16, name="sel", tag="sel")
        diag_D = pW3.tile([128, D2T * 128], BF16, name="diagD", tag="diagD")
        fc1T = pW3.tile([128, DC * H], BF16, name="fc1T", tag="fc1T")
        fc2T = pW3.tile([128, HT * D], BF16, name="fc2T", tag="fc2T")
        c_fc1 = pW3.tile([128, HT], F32, name="cfc1", tag="cfc1")
        fc2b = pW3.tile([1, D], F32R, name="fc2b", tag="fc2b")
        ones1 = pW3.tile([1, 128], F32R, name="ones1", tag="ones1")
        ident_bf = pW3.tile([128, 128], BF16, name="idbf", tag="idbf")
        ident_f = pW3.tile([128, 128], F32, name="idf", tag="idf")
        nc.sync.dma_start(out=ident_f[:, :], in_=din["ident_f"].ap()[:, :])
        nc.sync.dma_start(out=ident_bf[:, :], in_=din["ident_bf"].ap()[:, :])

        # phase1->3 tensors
        zh = [p13.tile([128, L], BF16, name=f"zh{i}", tag=f"zh{i}") for i in range(D2T)]
        y_cm = [p13.tile([128, L], BF16, name=f"ycm{i}", tag=f"ycm{i}") for i in range(D2T)]
        x_res = [p13.tile([128, D], BF16, name=f"xres{i}", tag=f"xres{i}") for i in range(TT)]
        # phase1->2 tensors
        xh = [p12.tile([128, L], BF16, name=f"xh{i}", tag=f"xh{i}") for i in range(D2T)]
        # dud[dt] = [du_h0 | dl_h0 | du_h1 | dl_h1] 512-blocks: one bcast DMA
        # per (group, half) carries du and delta together
        dud = [
            p12.tile([128, 2 * L], BF16, name=f"dud{i}", tag=f"dud{i}")
            for i in range(D2T)
        ]
        bbc = p12.tile([128, L], BF16, name="bbc", tag="bbc")
        cbc = p12.tile([128, L], BF16, name="cbc", tag="cbc")
        xdbl_dt = p12.tile([R, L], BF16, name="xdbl", tag="xdbl")
        bc_sb = p12.tile([2 * NS, L], BF16, name="bc_sb", tag="bc_sb")
        a_perm = p12.tile([128, NG], F32, name="aperm", tag="aperm")
        v_col = p12.tile([128, 1], F32, name="vcol", tag="vcol")
        dt_bias = p12.tile([128, D2T], F32, name="dtb", tag="dtb")
        c_in = p12.tile([128, ET], F32, name="cin", tag="cin")
        hfin = scanS.tile([128, NG], F32, name="hfin", tag="hfin")

        nc.sync.dma_start(out=c_in[:, :], in_=din["c_in"].ap()[:, :])
        eps_t = p12.tile([128, 1], F32, name="eps_t", tag="eps_t")
        nc.vector.memset(eps_t[:, :], EPS)

        # ---------- scan block: one (channel-tile, half) = 16 groups ----------
        # fillers: queue of closures emitting ~1-7us of independent work;
        # popped after each pair's sel matmuls so the in-order PE queue has
        # ready work adjacent to the dependency-stalled scan matmuls
        def scan_block(dt, h, fillers=None):
            ps_y = psY.tile([128, HF], F32, name="ps_y", tag="ps_y")
            nc.tensor.matmul(
                ps_y[:, :],
                diag_D[:, dt * 128 : (dt + 1) * 128],
                xh[dt][:, h * HF : (h + 1) * HF],
                start=True,
                stop=False,
            )
            for qp in range(8):
                q0 = 2 * qp
                bdl = scanS.tile([128, 4 * HF], BF16, name="bdl", tag="bdl", bufs=4)
                dA = scanS.tile([128, 2 * HF], F32, name="dA", tag="dA", bufs=3)
                dBu = scanS.tile([128, 2 * HF], BF16, name="dBu", tag="dBu", bufs=4)
                hs = scanS.tile([128, 2 * HF], BF16, name="hs", tag="hs", bufs=3)
                yt = dBu  # dBu dead after the scans; reuse for yt
                for g in range(2):
                    q = q0 + g
                    nc.sync.dma_start(
                        out=bdl[:, g * 2 * HF : (g + 1) * 2 * HF],
                        in_=dud[dt][q * 8 : (q + 1) * 8, h * 2 * HF : (h + 1) * 2 * HF]
                        .unsqueeze(1)
                        .broadcast_to([8, NS, 2 * HF]),
                    )
                bdl4 = bdl[:, :].rearrange("p (b l) -> p b l", b=4)
                if shared_scale:
                    nc.scalar.activation(
                        out=dA[:, :].rearrange("p (b l) -> p b l", b=2),
                        in_=bdl4[:, 1::2, :],
                        func=AF.Exp,
                        bias=0.0,
                        scale=v_col[:, 0:1],
                    )
                else:
                    for g in range(2):
                        G = dt * 16 + q0 + g
                        nc.scalar.activation(
                            out=dA[:, g * HF : (g + 1) * HF],
                            in_=bdl[:, g * 2 * HF + HF : (g + 1) * 2 * HF],
                            func=AF.Exp,
                            bias=0.0,
                            scale=a_perm[:, G : G + 1],
                        )
                dbu_eng = nc.vector
                dbu_eng.tensor_tensor(
                    out=dBu[:, :].rearrange("p (b l) -> p b l", b=2),
                    in0=bdl4[:, 0::2, :],
                    in1=bbc[:, h * HF : (h + 1) * HF]
                    .unsqueeze(1)
                    .broadcast_to([128, 2, HF]),
                    op=ALU.mult,
                )
                for g in range(2):
                    G = dt * 16 + q0 + g
                    init = 0.0 if h == 0 else hfin[:, G : G + 1]
                    nc.vector.tensor_tensor_scan(
                        hs[:, g * HF : (g + 1) * HF],
                        dA[:, g * HF : (g + 1) * HF],
                        dBu[:, g * HF : (g + 1) * HF],
                        init,
                        ALU.mult,
                        ALU.add,
                    )
                if h == 0:
                    nc.vector.tensor_copy(
                        hfin[:, dt * 16 + q0 : dt * 16 + q0 + 2],
                        hs[:, :].rearrange("p (g l) -> p g l", g=2)[:, :, HF - 1],
                    )
                yt_eng = nc.vector if qp % 4 == 0 else nc.gpsimd
                yt_eng.tensor_tensor(
                    out=yt[:, :],
                    in0=hs[:, :],
                    in1=cbc[:, h * HF : (h + 1) * HF]
                    .unsqueeze(1)
                    .broadcast_to([128, 2, HF]),
                    op=ALU.mult,
                )
                for g in range(2):
                    q = q0 + g
                    nc.tensor.matmul(
                        ps_y[:, :],
                        sel[:, q * 128 : (q + 1) * 128],
                        yt[:, g * HF : (g + 1) * HF],
                        start=False,
                        stop=(q == 15),
                    )
                if fillers:
                    fillers.popleft()()
            nc.scalar.copy(out=y_cm[dt][:, h * HF : (h + 1) * HF], in_=ps_y[:, :])

        # ================= P1a + window 1 ==========
        with (
            tc.tile_pool(name="wE", bufs=1) as wE,
            tc.tile_pool(name="xpP", bufs=1) as xpP,
            tc.tile_pool(name="t1", bufs=2) as t1,
            tc.tile_pool(name="ts", bufs=3) as ts,
            tc.tile_pool(name="tsp", bufs=1) as tsp,
            tc.tile_pool(name="xhatT_p", bufs=1) as xhatT_p,
            tc.tile_pool(name="psTr", bufs=1, space="PSUM") as psTr,
            tc.tile_pool(name="psIn", bufs=2, space="PSUM") as psIn,
            tc.tile_pool(name="psTiny", bufs=1, space="PSUM") as psTiny,
            tc.tile_pool(name="psConv", bufs=2, space="PSUM") as psConv,
        ):
            w_inT = wE.tile([128, DC * E], BF16, name="winT", tag="winT")
            diag = {}
            for br in ("x", "z"):
                diag[br] = wE.tile(
                    [128, D2T * KC * 128], BF16, name=f"diag{br}", tag=f"diag{br}"
                )
            x_projT = wE.tile([128, D2T * (R + 2 * NS)], BF16, name="xpj", tag="xpj")
            dt_projT = wE.tile([R, D2], BF16, name="dtpj", tag="dtpj")
            rep_b = wE.tile([2 * NS, 128], BF16, name="rep_b", tag="rep_b")
            rep_c = wE.tile([2 * NS, 128], BF16, name="rep_c", tag="rep_c")

            xhatT = [
                xhatT_p.tile([128, L], BF16, name=f"xhT{i}", tag=f"xhT{i}")
                for i in range(DC)
            ]

            # ---- LN1 (token-major) + transpose, all 8 token tiles ----
            # PE clock warmup: burn idle DMA-wait time on dummy matmuls so
            # in_proj runs at full clock.
            ps_w = psTiny.tile([128, 128], F32, name="ps_w", tag="ps_w")
            for _ in range(8):
                nc.tensor.matmul(
                    ps_w[:, :], ident_f[:, :], ident_f[:, :], start=True, stop=True
                )
            for tt in range(TT):
                x_t = x_res[tt]
                nc.sync.dma_start(out=x_t[:, :], in_=xin[tt * 128 : (tt + 1) * 128, :])
                stats = ts.tile([128, 6], F32, name="stats", tag="stats")
                nc.vector.bn_stats(out=stats[:, :], in_=x_t[:, :])
                mv = ts.tile([128, 2], F32, name="mv", tag="mv")
                nc.vector.bn_aggr(out=mv[:, :], in_=stats[:, :])
                sd = ts.tile([128, 1], F32, name="sd", tag="sd")
                nc.scalar.activation(
                    out=sd[:, :], in_=mv[:, 1:2], func=AF.Sqrt, bias=eps_t[:, :], scale=1.0
                )
                r_t = ts.tile([128, 1], F32, name="r_t", tag="r_t")
                nc.vector.reciprocal(out=r_t[:, :], in_=sd[:, :])
                xhat = t1.tile([128, D], BF16, name="xhat", tag="xhat")
                nc.vector.tensor_scalar(
                    out=xhat[:, :],
                    in0=x_t[:, :],
                    scalar1=mv[:, 0:1],
                    scalar2=r_t[:, :],
                    op0=ALU.subtract,
                    op1=ALU.mult,
                )
                # keep PE continuously busy between transposes (pstate)
                for _ in range(3):
                    nc.tensor.matmul(
                        ps_w[:, :], ident_f[:, :], ident_f[:, :], start=True, stop=True
                    )
                ps_tr = psTr.tile([128, D], BF16, name="ps_tr", tag="ps_tr")
                for dc in range(DC):
                    nc.tensor.transpose(
                        ps_tr[:, dc * 128 : (dc + 1) * 128],
                        xhat[:, dc * 128 : (dc + 1) * 128],
                        ident_bf[:, :],
                    )
                for dc in range(DC):
                    nc.vector.tensor_copy(
                        xhatT[dc][:, tt * 128 : (tt + 1) * 128],
                        ps_tr[:, dc * 128 : (dc + 1) * 128],
                    )

            # weight DMAs behind xin on the SP FIFO queue, ordered by first use
            nc.sync.dma_start(out=w_inT[:, :], in_=din["w_inT"].ap()[:, :])
            nc.sync.dma_start(out=diag["x"][:, :], in_=din["diag_x"].ap()[:, :])
            nc.sync.dma_start(out=x_projT[:, :], in_=din["x_projT"].ap()[:, :])
            nc.sync.dma_start(out=dt_projT[:, :], in_=din["dt_projT"].ap()[:, :])
            nc.sync.dma_start(out=rep_b[:, :], in_=din["rep_b"].ap()[:, :])
            nc.sync.dma_start(out=rep_c[:, :], in_=din["rep_c"].ap()[:, :])
            nc.sync.dma_start(out=dt_bias[:, :], in_=din["dt_bias"].ap()[:, :])
            nc.sync.dma_start(out=sel[:, :], in_=din["sel"].ap()[:, :])
            nc.sync.dma_start(out=a_perm[:, :], in_=din["A_perm"].ap()[:, :])
            nc.sync.dma_start(out=v_col[:, :], in_=din["v_col"].ap()[:, :])
            nc.sync.dma_start(out=diag_D[:, :], in_=din["diag_D"].ap()[:, :])
            nc.sync.dma_start(out=diag["z"][:, :], in_=din["diag_z"].ap()[:, :])
            nc.sync.dma_start(out=out_projT[:, :], in_=din["out_projT"].ap()[:, :])
            nc.sync.dma_start(out=c_fc1[:, :], in_=din["c_fc1"].ap()[:, :])
            nc.sync.dma_start(out=fc2b[:, :], in_=din["fc2b"].ap()[:, :])
            nc.sync.dma_start(out=ones1[:, :], in_=din["ones1d"].ap()[:, :])

            # ---- conv input buffers (padded by 1 left / 2 right) ----
            xp = {
                "x": [
                    xpP.tile([128, L + 3], BF16, name=f"xpx{i}", tag=f"xpx{i}")
                    for i in range(D2T)
                ],
                "z": [
                    xpP.tile([128, L + 3], BF16, name=f"xpz{i}", tag=f"xpz{i}")
                    for i in range(D2T)
                ],
            }
            for br in ("x", "z"):
                for dtc in range(D2T):
                    nc.sync.dma_start(out=xp[br][dtc][:, 0:1], in_=din["zpad"].ap()[:, 0:1])
                    nc.sync.dma_start(
                        out=xp[br][dtc][:, L + 1 : L + 3], in_=din["zpad"].ap()[:, 0:2]
                    )

            # in_proj token ranges: h0 covers [0,514) (conv lookahead), h1 the rest
            def in_proj_half(et, h):
                br, dtc = ("x", et) if et < D2T else ("z", et - D2T)
                chunks = [(0, 512), (512, 514)] if h == 0 else [(514, 1024)]
                for c0, c1 in chunks:
                    w = c1 - c0
                    if w > 16:
                        ps = psIn.tile([128, 512], F32, name="ps_inp", tag="ps_inp")
                    else:
                        ps = psTiny.tile([128, 128], F32, name="ps_w", tag="ps_w")
                    for dc in range(DC):
                        nc.tensor.matmul(
                            ps[:, 0:w],
                            w_inT[:, dc * E + et * 128 : dc * E + (et + 1) * 128],
                            xhatT[dc][:, c0:c1],
                            start=(dc == 0),
                            stop=(dc == DC - 1),
                        )
                    nc.vector.tensor_scalar(
                        out=xp[br][dtc][:, 1 + c0 : 1 + c1],
                        in0=ps[:, 0:w],
                        scalar1=c_in[:, et : et + 1],
                        scalar2=None,
                        op0=ALU.add,
                    )

            def conv_half(br, dtc, h, raw=False):
                ps = psConv.tile([128, 512], F32, name="ps_conv", tag="ps_conv")
                for j in range(KC):
                    nc.tensor.matmul(
                        ps[:, :],
                        diag[br][:, (dtc * KC + j) * 128 : (dtc * KC + j + 1) * 128],
                        xp[br][dtc][:, h * HF + j : h * HF + j + HF],
                        start=(j == 0),
                        stop=(j == KC - 1),
                    )
                dst = xh[dtc] if br == "x" else zh[dtc]
                if raw:
                    # table-free ACT Copy evac (ready straight from the PE
                    # psum, so the following in-place silu island coheres);
                    # silu applied in place later to avoid Exp<->Silu thrash
                    nc.scalar.copy(out=dst[:, h * HF : (h + 1) * HF], in_=ps[:, :])
                else:
                    nc.scalar.activation(
                        out=dst[:, h * HF : (h + 1) * HF],
                        in_=ps[:, :],
                        func=AF.Silu,
                        bias=0.0,
                        scale=1.0,
                    )

            def silu_island(specs):
                for br, dtc, h in specs:
                    dst = xh[dtc] if br == "x" else zh[dtc]
                    nc.scalar.activation(
                        out=dst[:, h * HF : (h + 1) * HF],
                        in_=dst[:, h * HF : (h + 1) * HF],
                        func=AF.Silu,
                        bias=0.0,
                        scale=1.0,
                    )

            def xproj_half(h):
                RW = R + 2 * NS
                ps = psIn.tile([128, 512], F32, name="ps_xd", tag="ps_inp")
                for dtc in range(D2T):
                    nc.tensor.matmul(
                        ps[0:RW, :],
                        x_projT[:, dtc * RW : (dtc + 1) * RW],
                        xh[dtc][:, h * HF : (h + 1) * HF],
                        start=(dtc == 0),
                        stop=(dtc == D2T - 1),
                    )
                nc.vector.tensor_copy(xdbl_dt[:, h * HF : (h + 1) * HF], ps[0:R, :])
                nc.vector.tensor_copy(
                    bc_sb[:, h * HF : (h + 1) * HF], ps[R : R + 2 * NS, :]
                )
                for dst_t, rep_t in ((bbc, rep_b), (cbc, rep_c)):
                    ps2 = psIn.tile([128, 512], F32, name="ps_bc", tag="ps_inp")
                    nc.tensor.matmul(
                        ps2[:, :],
                        rep_t[:, :],
                        bc_sb[:, h * HF : (h + 1) * HF],
                        start=True,
                        stop=True,
                    )
                    nc.vector.tensor_copy(dst_t[:, h * HF : (h + 1) * HF], ps2[:, :])

            def dt_soft_half(h, du_eng):
                # Exps batched before Lns (same ACT table set covers both)
                t_sps = []
                for dtc in range(D2T):
                    ps3 = psConv.tile([128, 512], F32, name="ps_dt", tag="ps_conv")
                    nc.tensor.matmul(
                        ps3[:, :],
                        dt_projT[:, dtc * 128 : (dtc + 1) * 128],
                        xdbl_dt[:, h * HF : (h + 1) * HF],
                        start=True,
                        stop=True,
                    )
                    t_sp = tsp.tile(
                        [128, 512], F32, name=f"tsp{dtc}", tag=f"tsp{dtc}", bufs=1
                    )
                    nc.scalar.activation(
                        out=t_sp[:, :],
                        in_=ps3[:, :],
                        func=AF.Exp,
                        bias=dt_bias[:, dtc : dtc + 1],
                        scale=1.0,
                    )
                    t_sps.append(t_sp)
                for dtc in range(D2T):
                    nc.scalar.activation(
                        out=dud[dtc][:, h * 2 * HF + HF : (h + 1) * 2 * HF],
                        in_=t_sps[dtc][:, :],
                        func=AF.Ln,
                        bias=1.0,
                        scale=1.0,
                    )
                    du_eng.tensor_tensor(
                        out=dud[dtc][:, h * 2 * HF : h * 2 * HF + HF],
                        in0=dud[dtc][:, h * 2 * HF + HF : (h + 1) * 2 * HF],
                        in1=xh[dtc][:, h * HF : (h + 1) * HF],
                        op=ALU.mult,
                    )

            # ---- P1a: everything the h0 scan needs ----
            for et in range(D2T):
                in_proj_half(et, 0)
            for dtc in range(D2T):
                conv_half("x", dtc, 0)
            xproj_half(0)
            dt_soft_half(0, nc.gpsimd)

            # ---- window 1: h0 scan blocks with P1b work in the pair slots ----
            from collections import deque

            # silu-bearing convs are merged into single items so their ACT
            # table loads happen once per island, not once per scan pair
            f1 = deque()
            for et in range(D2T):
                f1.append(lambda et=et: in_proj_half(et, 1))
            f1.append(lambda: [conv_half("x", d, 1, raw=True) for d in range(D2T)])
            f1.append(lambda: silu_island([("x", d, 1) for d in range(D2T)]))
            f1.append(lambda: xproj_half(1))
            f1.append(lambda: dt_soft_half(1, nc.gpsimd))
            for et in range(D2T, ET):
                f1.append(
                    lambda et=et: (in_proj_half(et, 0), in_proj_half(et, 1))
                )
            f1.append(
                lambda: [
                    conv_half("z", d, hh, raw=True)
                    for d in range(D2T)
                    for hh in range(2)
                ]
            )
            f1.append(
                lambda: silu_island(
                    [("z", d, hh) for d in range(D2T) for hh in range(2)]
                )
            )
            w1c = DC * H // 4
            for dt in range(D2T):
                scan_block(dt, 0, f1)
                nc.scalar.dma_start(
                    out=fc1T[:, dt * w1c : (dt + 1) * w1c],
                    in_=din["fc1T"].ap()[:, dt * w1c : (dt + 1) * w1c],
                )
            while f1:
                f1.popleft()()

        # ================= window 2 + phase 3 ==========
        with (
            tc.tile_pool(name="p3", bufs=1) as p3,
            tc.tile_pool(name="t3", bufs=2) as t3,
            tc.tile_pool(name="psG3", bufs=2, space="PSUM") as psG3,
            tc.tile_pool(name="psF1", bufs=2, space="PSUM") as psF1,
            tc.tile_pool(name="psTr3", bufs=1, space="PSUM") as psTr3,
        ):
            h_res = [
                p3.tile([128, D], F32, name=f"hres{i}", tag=f"hres{i}") for i in range(TT)
            ]
            xhat2 = [
                p3.tile([128, D], BF16, name=f"xh2{i}", tag=f"xh2{i}") for i in range(TT)
            ]
            xhat2T = [
                p3.tile([128, L], BF16, name=f"xh2T{i}", tag=f"xh2T{i}")
                for i in range(DC)
            ]
            # aT holds one L-half at a time: h0 is consumed by fc2(tt 0..3)
            # before fc1_half(1) overwrites it
            aT = [
                p3.tile([128, 512], BF16, name=f"aT{i}", tag=f"aT{i}")
                for i in range(HT)
            ]

            def out_proj_tt(tt):
                ps = psG3.tile([128, D], F32, name="ps_op", tag="g3")
                korder = list(range(D2T, ET)) + list(range(D2T))
                for ki, k in enumerate(korder):
                    lhs = (
                        y_cm[k][:, tt * 128 : (tt + 1) * 128]
                        if k < D2T
                        else zh[k - D2T][:, tt * 128 : (tt + 1) * 128]
                    )
                    nc.tensor.matmul(
                        ps[:, :],
                        lhs,
                        out_projT[:, k * D : (k + 1) * D],
                        start=(ki == 0),
                        stop=(ki == ET - 1),
                    )
                # + residual on DVE (also evacuates the psum)
                nc.vector.tensor_tensor(
                    out=h_res[tt][:, :], in0=ps[:, :], in1=x_res[tt][:, :], op=ALU.add
                )

            def ln2_stats_tt(tt):
                # ACT-side stats: Square+accum and Identity+accum are in every
                # ACT table set, so they don't thrash the Exp table mid-scan
                sq = t3.tile([128, D], F32, name="sq3", tag="sq3")
                qsum = t3.tile([128, 1], F32, name="qsum", tag="qsum", bufs=8)
                ssum = t3.tile([128, 1], F32, name="ssum", tag="ssum", bufs=8)
                nc.scalar.activation(
                    out=sq[:, :], in_=h_res[tt][:, :], func=AF.Square, accum_out=qsum[:, :]
                )
                nc.scalar.activation(
                    out=sq[:, :], in_=h_res[tt][:, :], func=AF.Identity,
                    accum_out=ssum[:, :],
                )
                return qsum, ssum

            def ln2_finish_batch(tts):
                parts = {}
                for tt in tts:
                    qsum, ssum = ln2_acc[tt]
                    m = t3.tile([128, 1], F32, name="m3", tag="m3", bufs=8)
                    nc.vector.tensor_scalar(
                        out=m[:, :], in0=ssum[:, :], scalar1=1.0 / D, scalar2=None,
                        op0=ALU.mult,
                    )
                    msq = t3.tile([128, 1], F32, name="msq", tag="msq", bufs=8)
                    nc.vector.tensor_tensor(
                        out=msq[:, :], in0=m[:, :], in1=m[:, :], op=ALU.mult
                    )
                    v = t3.tile([128, 1], F32, name="v3", tag="v3", bufs=8)
                    nc.vector.scalar_tensor_tensor(
                        out=v[:, :], in0=qsum[:, :], scalar=1.0 / D, in1=msq[:, :],
                        op0=ALU.mult, op1=ALU.subtract,
                    )
                    parts[tt] = (m, v)
                sds = {}
                for tt in tts:
                    m, v = parts[tt]
                    sd = t3.tile([128, 1], F32, name="sd3", tag="sd3", bufs=8)
                    nc.scalar.activation(
                        out=sd[:, :], in_=v[:, :], func=AF.Sqrt, bias=eps_t[:, :],
                        scale=1.0,
                    )
                    sds[tt] = sd
                for tt in tts:
                    m, v = parts[tt]
                    r_t = t3.tile([128, 1], F32, name="r3", tag="r3", bufs=8)
                    nc.vector.reciprocal(out=r_t[:, :], in_=sds[tt][:, :])
                    nc.vector.tensor_scalar(
                        out=xhat2[tt][:, :],
                        in0=h_res[tt][:, :],
                        scalar1=m[:, 0:1],
                        scalar2=r_t[:, :],
                        op0=ALU.subtract,
                        op1=ALU.mult,
                    )

            def ln2_finish_tt(tt, qsum, ssum):
                # m = ssum/512; v = qsum/512 - m^2; r = 1/sqrt(v+eps); all tiny
                m = t3.tile([128, 1], F32, name="m3", tag="m3")
                nc.vector.tensor_scalar(
                    out=m[:, :], in0=ssum[:, :], scalar1=1.0 / D, scalar2=None,
                    op0=ALU.mult,
                )
                # v = qsum/512 - m*m
                msq = t3.tile([128, 1], F32, name="msq", tag="msq")
                nc.vector.tensor_tensor(
                    out=msq[:, :], in0=m[:, :], in1=m[:, :], op=ALU.mult
                )
                v = t3.tile([128, 1], F32, name="v3", tag="v3")
                nc.vector.scalar_tensor_tensor(
                    out=v[:, :], in0=qsum[:, :], scalar=1.0 / D, in1=msq[:, :],
                    op0=ALU.mult, op1=ALU.subtract,
                )
                sd = t3.tile([128, 1], F32, name="sd3", tag="sd3")
                nc.scalar.activation(
                    out=sd[:, :], in_=v[:, :], func=AF.Sqrt, bias=eps_t[:, :], scale=1.0
                )
                r_t = t3.tile([128, 1], F32, name="r3", tag="r3")
                nc.vector.reciprocal(out=r_t[:, :], in_=sd[:, :])
                nc.vector.tensor_scalar(
                    out=xhat2[tt][:, :],
                    in0=h_res[tt][:, :],
                    scalar1=m[:, 0:1],
                    scalar2=r_t[:, :],
                    op0=ALU.subtract,
                    op1=ALU.mult,
                )

            def tr3_dc(half, dc):
                ps_t = psTr3.tile([128, 512], BF16, name="ps_t3", tag="ps_t3")
                for b4 in range(4):
                    tt = half * 4 + b4
                    nc.tensor.transpose(
                        ps_t[:, b4 * 128 : (b4 + 1) * 128],
                        xhat2[tt][:, dc * 128 : (dc + 1) * 128],
                        ident_bf[:, :],
                    )
                nc.scalar.copy(
                    out=xhat2T[dc][:, half * 512 : (half + 1) * 512], in_=ps_t[:, :]
                )

            def fc1_block(half, ht0, ht1, raw=False):
                for ht in range(ht0, ht1):
                    ps = psF1.tile([128, 512], F32, name="ps_fc1", tag="f1")
                    for dc in range(DC):
                        nc.tensor.matmul(
                            ps[:, :],
                            fc1T[:, dc * H + ht * 128 : dc * H + (ht + 1) * 128],
                            xhat2T[dc][:, half * 512 : (half + 1) * 512],
                            start=(dc == 0),
                            stop=(dc == DC - 1),
                        )
                    if raw:
                        nc.scalar.copy(out=aT[ht][:, :], in_=ps[:, :])
                    else:
                        nc.scalar.activation(
                            out=aT[ht][:, :],
                            in_=ps[:, :],
                            func=AF.Gelu,
                            bias=c_fc1[:, ht : ht + 1],
                            scale=1.0,
                        )

            def gelu_island():
                for ht in range(HT):
                    nc.scalar.activation(
                        out=aT[ht][:, :],
                        in_=aT[ht][:, :],
                        func=AF.Gelu,
                        bias=c_fc1[:, ht : ht + 1],
                        scale=1.0,
                    )

            def fc2_tt(tt):
                ps = psG3.tile([128, D], F32, name="ps_fc2", tag="g3")
                for ht in range(HT):
                    nc.tensor.matmul(
                        ps[:, :],
                        aT[ht][:, (tt % 4) * 128 : (tt % 4 + 1) * 128],
                        fc2T[:, ht * D : (ht + 1) * D],
                        start=(ht == 0),
                        stop=False,
                    )
                nc.tensor.matmul(
                    ps[:, :], ones1[:, :], fc2b[:, :], start=False, stop=True
                )
                o_t = t3.tile([128, D], F32, name="o_t", tag="o_t")
                nc.vector.tensor_tensor(
                    out=o_t[:, :], in0=ps[:, :], in1=h_res[tt][:, :], op=ALU.add
                )
                nc.sync.dma_start(
                    out=out_ap[tt * 128 : (tt + 1) * 128, :], in_=o_t[:, :]
                )

            # ---- window 2: h1 scan blocks with phase-3 h0 in the pair slots ----
            from collections import deque

            ln2_acc = {}

            def op_item(tt):
                out_proj_tt(tt)
                ln2_acc[tt] = ln2_stats_tt(tt)

            def ln2_batch(tts):
                ln2_finish_batch(list(tts))

            f2 = deque()
            for tt in range(4):
                f2.append(lambda t=tt: op_item(t))
            f2.append(lambda: ln2_batch(range(4)))
            for dc in range(DC):
                f2.append(lambda d=dc: tr3_dc(0, d))
            # fc1 evacs are raw copies; one in-place gelu island bounds the
            # ACT table switches to a single Exp<->Gelu round trip
            f2.append(lambda: fc1_block(0, 0, 8, raw=True))
            f2.append(lambda: fc1_block(0, 8, 16, raw=True))
            f2.append(gelu_island)
            for tt in range(4):
                f2.append(lambda t=tt: fc2_tt(t))

            # fc2T must be emitted before the fc2 filler items (tile deps
            # bind at emission order); stream in two chunks up front
            w2c = HT * D // 2
            for ck in range(2):
                nc.scalar.dma_start(
                    out=fc2T[:, ck * w2c : (ck + 1) * w2c],
                    in_=din["fc2T"].ap()[:, ck * w2c : (ck + 1) * w2c],
                )
            for dt in range(D2T):
                scan_block(dt, 1, f2)

            # ---- tail: phase-3 for h1 ----
            while f2:
                f2.popleft()()
            for tt in range(4, TT):
                op_item(tt)
            ln2_batch(range(4, TT))
            for dc in range(DC):
                tr3_dc(1, dc)
            fc1_block(1, 0, 16)
            for tt in range(4, TT):
                fc2_tt(tt)


def _mk_repbc(row0):
    m = np.zeros((2 * NS, 128), np.float32)
    p = np.arange(128)
    m[row0 + (p % 16), p] = 1.0
    return m


def prep_inputs(inputs):
    """Host-side weight preprocessing. Returns the shared (non-x) in_map."""
    g = {k: np.asarray(v, dtype=np.float32) for k, v in inputs.items()}

    ln1_w, ln1_b = g["ln1_w"], g["ln1_b"]
    ln2_w, ln2_b = g["ln2_w"], g["ln2_b"]

    w_in = g["in_proj_w"] * ln1_w[None, :]  # [E, D]
    c_in = (g["in_proj_w"] @ ln1_b).astype(np.float32)  # [E]

    fc1 = g["fc1_w"] * ln2_w[None, :]  # [H, D]
    c_fc1 = (g["fc1_w"] @ ln2_b + g["fc1_b"]).astype(np.float32)  # [H]

    A = -np.exp(g["A_log"])  # [D2, NS]

    # REP[q][k, m] = 1 iff k == q*8 + m//16   (delta row broadcast)
    rep = np.zeros((16, 128, 128), np.float32)
    for q in range(16):
        m = np.arange(128)
        rep[q, q * 8 + m // 16, m] = 1.0
    # SEL[q][k, m] = 1 iff m == q*8 + k//16   (sum over n into channel rows)
    sel = np.transpose(rep, (0, 2, 1)).copy()
    # A_perm[p, g] = A[g*8 + p//16, p%16]
    p = np.arange(128)
    gg = np.arange(NG)
    A_perm = A[(gg[None, :] * 8 + (p // 16)[:, None]), (p % 16)[:, None]].astype(
        np.float32
    )
    # shared-scale fast path: A rows identical across channels (A[c,n] = v[n])
    v = A[0, :]
    a_shared = bool(np.allclose(A, v[None, :], rtol=1e-5, atol=1e-7))
    v_col = v[(p % 16)].reshape(128, 1).astype(np.float32)
    diag_D = np.zeros((D2T, 128, 128), np.float32)
    idx128 = np.arange(128)
    for dt in range(D2T):
        diag_D[dt, idx128, idx128] = g["ssm_D"][dt * 128 : (dt + 1) * 128]

    conv_x = g["conv_x_w"][:, 0, :]  # [D2, KC]
    conv_z = g["conv_z_w"][:, 0, :]
    diag_x = np.zeros((D2T * KC, 128, 128), np.float32)
    diag_z = np.zeros((D2T * KC, 128, 128), np.float32)
    idx = np.arange(128)
    for dt in range(D2T):
        for j in range(KC):
            diag_x[dt * KC + j, idx, idx] = conv_x[dt * 128 : (dt + 1) * 128, j]
            diag_z[dt * KC + j, idx, idx] = conv_z[dt * 128 : (dt + 1) * 128, j]

    def bf(x):
        return np.ascontiguousarray(x.astype(_BF))

    def f(x):
        return np.ascontiguousarray(x.astype(np.float32))

    def blocks(arr3):  # [N, 128, W] -> [128, N*W]
        n, pdim, w = arr3.shape
        assert pdim == 128
        return np.ascontiguousarray(arr3.transpose(1, 0, 2).reshape(128, n * w))

    RW = R + 2 * NS
    shared = {
        "w_inT": bf(blocks(w_in.T.reshape(DC, 128, E))),
        "c_in": f(c_in.reshape(ET, 128).T),
        "diag_x": bf(blocks(diag_x)),
        "diag_z": bf(blocks(diag_z)),
        "x_projT": bf(blocks(g["x_proj_w"].T.reshape(D2T, 128, RW))),
        "dt_projT": bf(g["dt_proj_w"].T),
        "dt_bias": f(g["dt_proj_b"].reshape(D2T, 128).T),
        "A_perm": f(A_perm),
        "v_col": f(v_col),
        "diag_D": bf(blocks(diag_D)),
        "sel": bf(blocks(sel)),
        "out_projT": bf(blocks(g["out_proj_w"].T.reshape(ET, 128, D))),
        "fc1T": bf(blocks(fc1.T.reshape(DC, 128, H))),
        "c_fc1": f(c_fc1.reshape(HT, 128).T),
        "fc2T": bf(blocks(g["fc2_w"].T.reshape(HT, 128, D))),
        "fc2b": f(g["fc2_b"].reshape(1, D)),
        "ident_bf": bf(np.eye(128, dtype=np.float32)),
        "zpad": np.zeros((128, 3), _BF),
        "ones1d": np.ones((1, 128), np.float32),
        "rep_b": bf(_mk_repbc(0)),
        "rep_c": bf(_mk_repbc(NS)),
        "ident_f": np.eye(128, dtype=np.float32),
    }
    return shared, a_shared


_CACHED_NC = {}


def kernel(**inputs):
    from concourse.bass_utils import run_bass_kernel_spmd

    shared, a_shared = prep_inputs(inputs)
    if a_shared not in _CACHED_NC:
        _CACHED_NC[a_shared] = build_kernel(shared_scale=a_shared)
    nc = _CACHED_NC[a_shared]
    x = np.asarray(inputs["x"], dtype=np.float32)
    in_maps = [
        dict(
            shared,
            xin=np.ascontiguousarray(
                np.concatenate([x[i]] * KREPEAT, axis=0).astype(_BF)
            ),
        )
        for i in range(NCORES)
    ]
    res = run_bass_kernel_spmd(nc, in_maps, core_ids=list(range(NCORES)))
    out = np.stack([res.results[i]["out"][:L] for i in range(NCORES)], axis=0)
    return out


if __name__ == "__main__":
    nc = build_kernel()
    print("build ok")



# revision 37
# speedup vs baseline: 1.0435x; 1.0432x over previous
"""Trainium2 Bass kernel for nn_Block_16621523436203 (Mamba-style block).

Sharding: pure data-parallel — batch B=8, one batch element per NeuronCore,
no collectives.  Weights are preprocessed (transposed / LN-folded / cast) on
host; each core runs the full block for its batch element.

Engine plan (per core).  HW constraints found the hard way: GPSIMD (Pool)
cannot touch PSUM and only runs plain TensorTensor (0.42 eff) + DMA;
tensor_tensor_scan is DVE-only; engine outputs consumed by f32r matmuls
must be written with f32r out-dtype; Memset cannot write f32r.

  P1  LN1 -> in_proj -> dwconv+SiLU (x and z) -> x_proj -> dt_proj/softplus
      PE: matmuls+transposes (bf16 weights), ACT: rsqrt(=Sqrt+recip)/SiLU/
      softplus (Exps then Lns batched for table locality), DVE: LN stats,
      evacuations, du mult.  z branch is emitted last so it overlaps the
      scan startup.  Weight DMAs are ordered by first use behind xin on the
      SP HWDGE queue.
  P2  selective scan, 64 groups of (8ch x 16st) partitions, processed in
      pairs ([128, 2048] tiles amortize fixed op cost):
      PE: delta-broadcast matmul (f32r) + D*u diag matmul + n-reduction
      matmul, ACT: exp(dA) + y_cm psum evac, DMA: du broadcast,
      Pool: dBu = du_bc*B mults, DVE: all scans + yt = hs*C mults.
      fc1/fc2 weights stream in via the ACT HWDGE queue during the scan.
  P3  out_proj -> +x -> LN2 -> (transpose -> fc1+GELU -> fc2 + residual)
      pipelined in L-halves; PE-bound.
ACT function-table thrash is minimized (Sqrt/Silu/Exp/Ln/Gelu runs).
"""

import sys

sys.path.insert(0, "/opt/trn_rl_repo")

import os

import ml_dtypes
import numpy as np

import concourse.bacc as bacc
import concourse.bass as bass
import concourse.mybir as mybir
import concourse.tile as tile

F32 = mybir.dt.float32
F32R = mybir.dt.float32r
BF16 = mybir.dt.bfloat16
AF = mybir.ActivationFunctionType
ALU = mybir.AluOpType

B, L, D = 8, 1024, 512
E = 1024  # d_inner
D2 = 512  # per-branch channels
R = 32  # dt_rank
NS = 16  # d_state
KC = 4  # conv kernel size
H = 2048  # mlp hidden
NCORES = 8
TT = L // 128  # 8 token tiles
DC = D // 128  # 4 d_model chunks
D2T = D2 // 128  # 4 channel tiles
ET = E // 128  # 8 d_inner tiles
HT = H // 128  # 16 hidden tiles
NG = 64  # scan groups: each = 8 channels x 16 states
EPS = 1e-5

_BF = ml_dtypes.bfloat16


def _f32r(ap):
    return ap.bitcast(F32R)


STOP_AFTER = int(os.environ.get("KSTOP", "3"))
KREPEAT = int(os.environ.get("KREPEAT", "1"))
KALLOC = int(os.environ.get("KALLOC", "0")) or KREPEAT


def build_kernel(shared_scale=True):
    nc = bacc.Bacc("TRN2", target_bir_lowering=False, debug=False, num_devices=1)

    din = {}

    def inp(name, shape, dtype):
        din[name] = nc.dram_tensor(name, list(shape), dtype, kind="ExternalInput")
        return din[name]

    inp("xin", (KALLOC * L, D), BF16)
    inp("w_inT", (128, DC * E), BF16)  # ln1-folded in_proj weight, d-major blocks
    inp("c_in", (128, ET), F32)  # in_proj bias column per e-tile (W' @ ln1_b)
    inp("diag_x", (128, D2T * KC * 128), BF16)  # conv diag matrices side by side
    inp("diag_z", (128, D2T * KC * 128), BF16)
    inp("x_projT", (128, D2T * (R + 2 * NS)), BF16)
    inp("dt_projT", (R, D2), BF16)
    inp("dt_bias", (128, D2T), F32)
    inp("A_perm", (128, NG), F32)  # A[d(p), n(p)] per group column
    inp("v_col", (128, 1), F32)  # shared exp scale when A is rank-1 (A[c,n]=v[n])
    inp("diag_D", (128, D2T * 128), BF16)  # diag(D) per channel tile
    inp("sel", (128, 16 * 128), BF16)  # n-reduction matmul: SEL[q] blocks
    inp("out_projT", (128, ET * D), BF16)
    inp("fc1T", (128, DC * H), BF16)  # ln2-folded fc1 weight
    inp("c_fc1", (128, HT), F32)  # fc1' @ ln2_b + fc1_b per h-tile
    inp("fc2T", (128, HT * D), BF16)
    inp("fc2b", (1, D), F32R)
    inp("ident_bf", (128, 128), BF16)
    inp("zpad", (128, 3), BF16)
    inp("ones1d", (1, 128), F32R)
    inp("rep_b", (2 * NS, 128), BF16)
    inp("rep_c", (2 * NS, 128), BF16)
    inp("ident_f", (128, 128), F32)

    out_d = nc.dram_tensor("out", [KALLOC * L, D], F32, kind="ExternalOutput")

    with tile.TileContext(nc) as tc:
        for rep_i in range(KREPEAT):
            _body(nc, tc, din, out_d, rep_i * L, shared_scale=shared_scale)
    nc.compile()
    return nc


def _body(nc, tc, din, out_d, row0=0, shared_scale=True):
    xin = din["xin"].ap()[row0 : row0 + L, :]
    out_ap = out_d.ap()[row0 : row0 + L, :]
    HF = L // 2  # half length; the scan is chained across halves so that
    # phase-1 (h1) and phase-3 (h0) matmul work overlaps the DVE-bound scan

    with (
        tc.tile_pool(name="pW3", bufs=1) as pW3,  # weights alive to the end
        tc.tile_pool(name="p13", bufs=1) as p13,  # crosses into phase 3
        tc.tile_pool(name="p12", bufs=1) as p12,  # dies after the scan
        tc.tile_pool(name="scanS", bufs=1) as scanS,  # scan tiles, both windows
        tc.tile_pool(name="psY", bufs=2, space="PSUM") as psY,
    ):
        out_projT = pW3.tile([128, ET * D], BF16, name="opT", tag="opT")
        sel = pW3.tile([128, 16 * 128], BF16, name="sel", tag="sel")
        diag_D = pW3.tile([128, D2T * 128], BF16, name="diagD", tag="diagD")
        fc1T = pW3.tile([128, DC * H], BF16, name="fc1T", tag="fc1T")
        fc2T = pW3.tile([128, HT * D], BF16, name="fc2T", tag="fc2T")
        c_fc1 = pW3.tile([128, HT], F32, name="cfc1", tag="cfc1")
        fc2b = pW3.tile([1, D], F32R, name="fc2b", tag="fc2b")
        ones1 = pW3.tile([1, 128], F32R, name="ones1", tag="ones1")
        ident_bf = pW3.tile([128, 128], BF16, name="idbf", tag="idbf")
        ident_f = pW3.tile([128, 128], F32, name="idf", tag="idf")
        nc.sync.dma_start(out=ident_f[:, :], in_=din["ident_f"].ap()[:, :])
        nc.sync.dma_start(out=ident_bf[:, :], in_=din["ident_bf"].ap()[:, :])

        # phase1->3 tensors
        zh = [p13.tile([128, L], BF16, name=f"zh{i}", tag=f"zh{i}") for i in range(D2T)]
        y_cm = [p13.tile([128, L], BF16, name=f"ycm{i}", tag=f"ycm{i}") for i in range(D2T)]
        x_res = [p13.tile([128, D], BF16, name=f"xres{i}", tag=f"xres{i}") for i in range(TT)]
        # phase1->2 tensors
        xh = [p12.tile([128, L], BF16, name=f"xh{i}", tag=f"xh{i}") for i in range(D2T)]
        # dud[dt] = [du_h0 | dl_h0 | du_h1 | dl_h1] 512-blocks: one bcast DMA
        # per (group, half) carries du and delta together
        dud = [
            p12.tile([128, 2 * L], BF16, name=f"dud{i}", tag=f"dud{i}")
            for i in range(D2T)
        ]
        bbc = p12.tile([128, L], BF16, name="bbc", tag="bbc")
        cbc = p12.tile([128, L], BF16, name="cbc", tag="cbc")
        xdbl_dt = p12.tile([R, L], BF16, name="xdbl", tag="xdbl")
        bc_sb = p12.tile([2 * NS, L], BF16, name="bc_sb", tag="bc_sb")
        a_perm = p12.tile([128, NG], F32, name="aperm", tag="aperm")
        v_col = p12.tile([128, 1], F32, name="vcol", tag="vcol")
        dt_bias = p12.tile([128, D2T], F32, name="dtb", tag="dtb")
        c_in = p12.tile([128, ET], F32, name="cin", tag="cin")
        hfin = scanS.tile([128, NG], F32, name="hfin", tag="hfin")

        nc.sync.dma_start(out=c_in[:, :], in_=din["c_in"].ap()[:, :])
        eps_t = p12.tile([128, 1], F32, name="eps_t", tag="eps_t")
        nc.vector.memset(eps_t[:, :], EPS)

        # ---------- scan block: one (channel-tile, half) = 16 groups ----------
        # fillers: queue of closures emitting ~1-7us of independent work;
        # popped after each pair's sel matmuls so the in-order PE queue has
        # ready work adjacent to the dependency-stalled scan matmuls
        def scan_block(dt, h, fillers=None):
            ps_y = psY.tile([128, HF], F32, name="ps_y", tag="ps_y")
            nc.tensor.matmul(
                ps_y[:, :],
                diag_D[:, dt * 128 : (dt + 1) * 128],
                xh[dt][:, h * HF : (h + 1) * HF],
                start=True,
                stop=False,
            )
            for qp in range(8):
                q0 = 2 * qp
                bdl = scanS.tile([128, 4 * HF], BF16, name="bdl", tag="bdl", bufs=4)
                dA = scanS.tile([128, 2 * HF], F32, name="dA", tag="dA", bufs=3)
                dBu = scanS.tile([128, 2 * HF], BF16, name="dBu", tag="dBu", bufs=4)
                hs = scanS.tile([128, 2 * HF], BF16, name="hs", tag="hs", bufs=3)
                yt = dBu  # dBu dead after the scans; reuse for yt
                for g in range(2):
                    q = q0 + g
                    nc.sync.dma_start(
                        out=bdl[:, g * 2 * HF : (g + 1) * 2 * HF],
                        in_=dud[dt][q * 8 : (q + 1) * 8, h * 2 * HF : (h + 1) * 2 * HF]
                        .unsqueeze(1)
                        .broadcast_to([8, NS, 2 * HF]),
                    )
                bdl4 = bdl[:, :].rearrange("p (b l) -> p b l", b=4)
                if shared_scale:
                    nc.scalar.activation(
                        out=dA[:, :].rearrange("p (b l) -> p b l", b=2),
                        in_=bdl4[:, 1::2, :],
                        func=AF.Exp,
                        bias=0.0,
                        scale=v_col[:, 0:1],
                    )
                else:
                    for g in range(2):
                        G = dt * 16 + q0 + g
                        nc.scalar.activation(
                            out=dA[:, g * HF : (g + 1) * HF],
                            in_=bdl[:, g * 2 * HF + HF : (g + 1) * 2 * HF],
                            func=AF.Exp,
                            bias=0.0,
                            scale=a_perm[:, G : G + 1],
                        )
                dbu_eng = nc.vector
                dbu_eng.tensor_tensor(
                    out=dBu[:, :].rearrange("p (b l) -> p b l", b=2),
                    in0=bdl4[:, 0::2, :],
                    in1=bbc[:, h * HF : (h + 1) * HF]
                    .unsqueeze(1)
                    .broadcast_to([128, 2, HF]),
                    op=ALU.mult,
                )
                for g in range(2):
                    G = dt * 16 + q0 + g
                    init = 0.0 if h == 0 else hfin[:, G : G + 1]
                    nc.vector.tensor_tensor_scan(
                        hs[:, g * HF : (g + 1) * HF],
                        dA[:, g * HF : (g + 1) * HF],
                        dBu[:, g * HF : (g + 1) * HF],
                        init,
                        ALU.mult,
                        ALU.add,
                    )
                if h == 0:
                    nc.vector.tensor_copy(
                        hfin[:, dt * 16 + q0 : dt * 16 + q0 + 2],
                        hs[:, :].rearrange("p (g l) -> p g l", g=2)[:, :, HF - 1],
                    )
                yt_eng = nc.vector if qp % 4 == 0 else nc.gpsimd
                yt_eng.tensor_tensor(
                    out=yt[:, :],
                    in0=hs[:, :],
                    in1=cbc[:, h * HF : (h + 1) * HF]
                    .unsqueeze(1)
                    .broadcast_to([128, 2, HF]),
                    op=ALU.mult,
                )
                for g in range(2):
                    q = q0 + g
                    nc.tensor.matmul(
                        ps_y[:, :],
                        sel[:, q * 128 : (q + 1) * 128],
                        yt[:, g * HF : (g + 1) * HF],
                        start=False,
                        stop=(q == 15),
                    )
                if fillers:
                    fillers.popleft()()
            nc.scalar.copy(out=y_cm[dt][:, h * HF : (h + 1) * HF], in_=ps_y[:, :])

        # ================= P1a + window 1 ==========
        with (
            tc.tile_pool(name="wE", bufs=1) as wE,
            tc.tile_pool(name="xpP", bufs=1) as xpP,
            tc.tile_pool(name="t1", bufs=2) as t1,
            tc.tile_pool(name="ts", bufs=3) as ts,
            tc.tile_pool(name="tsp", bufs=1) as tsp,
            tc.tile_pool(name="xhatT_p", bufs=1) as xhatT_p,
            tc.tile_pool(name="psTr", bufs=1, space="PSUM") as psTr,
            tc.tile_pool(name="psIn", bufs=2, space="PSUM") as psIn,
            tc.tile_pool(name="psTiny", bufs=1, space="PSUM") as psTiny,
            tc.tile_pool(name="psConv", bufs=2, space="PSUM") as psConv,
        ):
            w_inT = wE.tile([128, DC * E], BF16, name="winT", tag="winT")
            diag = {}
            for br in ("x", "z"):
                diag[br] = wE.tile(
                    [128, D2T * KC * 128], BF16, name=f"diag{br}", tag=f"diag{br}"
                )
            x_projT = wE.tile([128, D2T * (R + 2 * NS)], BF16, name="xpj", tag="xpj")
            dt_projT = wE.tile([R, D2], BF16, name="dtpj", tag="dtpj")
            rep_b = wE.tile([2 * NS, 128], BF16, name="rep_b", tag="rep_b")
            rep_c = wE.tile([2 * NS, 128], BF16, name="rep_c", tag="rep_c")

            xhatT = [
                xhatT_p.tile([128, L], BF16, name=f"xhT{i}", tag=f"xhT{i}")
                for i in range(DC)
            ]

            # ---- LN1 (token-major) + transpose, all 8 token tiles ----
            # PE clock warmup: burn idle DMA-wait time on dummy matmuls so
            # in_proj runs at full clock.
            ps_w = psTiny.tile([128, 128], F32, name="ps_w", tag="ps_w")
            for _ in range(8):
                nc.tensor.matmul(
                    ps_w[:, :], ident_f[:, :], ident_f[:, :], start=True, stop=True
                )
            for tt in range(TT):
                x_t = x_res[tt]
                nc.sync.dma_start(out=x_t[:, :], in_=xin[tt * 128 : (tt + 1) * 128, :])
                stats = ts.tile([128, 6], F32, name="stats", tag="stats")
                nc.vector.bn_stats(out=stats[:, :], in_=x_t[:, :])
                mv = ts.tile([128, 2], F32, name="mv", tag="mv")
                nc.vector.bn_aggr(out=mv[:, :], in_=stats[:, :])
                sd = ts.tile([128, 1], F32, name="sd", tag="sd")
                nc.scalar.activation(
                    out=sd[:, :], in_=mv[:, 1:2], func=AF.Sqrt, bias=eps_t[:, :], scale=1.0
                )
                r_t = ts.tile([128, 1], F32, name="r_t", tag="r_t")
                nc.vector.reciprocal(out=r_t[:, :], in_=sd[:, :])
                xhat = t1.tile([128, D], BF16, name="xhat", tag="xhat")
                nc.vector.tensor_scalar(
                    out=xhat[:, :],
                    in0=x_t[:, :],
                    scalar1=mv[:, 0:1],
                    scalar2=r_t[:, :],
                    op0=ALU.subtract,
                    op1=ALU.mult,
                )
                # keep PE continuously busy between transposes (pstate)
                for _ in range(3):
                    nc.tensor.matmul(
                        ps_w[:, :], ident_f[:, :], ident_f[:, :], start=True, stop=True
                    )
                ps_tr = psTr.tile([128, D], BF16, name="ps_tr", tag="ps_tr")
                for dc in range(DC):
                    nc.tensor.transpose(
                        ps_tr[:, dc * 128 : (dc + 1) * 128],
                        xhat[:, dc * 128 : (dc + 1) * 128],
                        ident_bf[:, :],
                    )
                for dc in range(DC):
                    nc.vector.tensor_copy(
                        xhatT[dc][:, tt * 128 : (tt + 1) * 128],
                        ps_tr[:, dc * 128 : (dc + 1) * 128],
                    )

            # weight DMAs behind xin on the SP FIFO queue, ordered by first use
            nc.sync.dma_start(out=w_inT[:, :], in_=din["w_inT"].ap()[:, :])
            nc.sync.dma_start(out=diag["x"][:, :], in_=din["diag_x"].ap()[:, :])
            nc.sync.dma_start(out=x_projT[:, :], in_=din["x_projT"].ap()[:, :])
            nc.sync.dma_start(out=dt_projT[:, :], in_=din["dt_projT"].ap()[:, :])
            nc.sync.dma_start(out=rep_b[:, :], in_=din["rep_b"].ap()[:, :])
            nc.sync.dma_start(out=rep_c[:, :], in_=din["rep_c"].ap()[:, :])
            nc.sync.dma_start(out=dt_bias[:, :], in_=din["dt_bias"].ap()[:, :])
            nc.sync.dma_start(out=sel[:, :], in_=din["sel"].ap()[:, :])
            nc.sync.dma_start(out=a_perm[:, :], in_=din["A_perm"].ap()[:, :])
            nc.sync.dma_start(out=v_col[:, :], in_=din["v_col"].ap()[:, :])
            nc.sync.dma_start(out=diag_D[:, :], in_=din["diag_D"].ap()[:, :])
            nc.sync.dma_start(out=diag["z"][:, :], in_=din["diag_z"].ap()[:, :])
            nc.sync.dma_start(out=out_projT[:, :], in_=din["out_projT"].ap()[:, :])
            nc.sync.dma_start(out=c_fc1[:, :], in_=din["c_fc1"].ap()[:, :])
            nc.sync.dma_start(out=fc2b[:, :], in_=din["fc2b"].ap()[:, :])
            nc.sync.dma_start(out=ones1[:, :], in_=din["ones1d"].ap()[:, :])

            # ---- conv input buffers (padded by 1 left / 2 right) ----
            xp = {
                "x": [
                    xpP.tile([128, L + 3], BF16, name=f"xpx{i}", tag=f"xpx{i}")
                    for i in range(D2T)
                ],
                "z": [
                    xpP.tile([128, L + 3], BF16, name=f"xpz{i}", tag=f"xpz{i}")
                    for i in range(D2T)
                ],
            }
            for br in ("x", "z"):
                for dtc in range(D2T):
                    nc.sync.dma_start(out=xp[br][dtc][:, 0:1], in_=din["zpad"].ap()[:, 0:1])
                    nc.sync.dma_start(
                        out=xp[br][dtc][:, L + 1 : L + 3], in_=din["zpad"].ap()[:, 0:2]
                    )

            # in_proj token ranges: h0 covers [0,514) (conv lookahead), h1 the rest
            def in_proj_half(et, h):
                br, dtc = ("x", et) if et < D2T else ("z", et - D2T)
                chunks = [(0, 512), (512, 514)] if h == 0 else [(514, 1024)]
                for c0, c1 in chunks:
                    w = c1 - c0
                    if w > 16:
                        ps = psIn.tile([128, 512], F32, name="ps_inp", tag="ps_inp")
                    else:
                        ps = psTiny.tile([128, 128], F32, name="ps_w", tag="ps_w")
                    for dc in range(DC):
                        nc.tensor.matmul(
                            ps[:, 0:w],
                            w_inT[:, dc * E + et * 128 : dc * E + (et + 1) * 128],
                            xhatT[dc][:, c0:c1],
                            start=(dc == 0),
                            stop=(dc == DC - 1),
                        )
                    nc.vector.tensor_scalar(
                        out=xp[br][dtc][:, 1 + c0 : 1 + c1],
                        in0=ps[:, 0:w],
                        scalar1=c_in[:, et : et + 1],
                        scalar2=None,
                        op0=ALU.add,
                    )

            def conv_half(br, dtc, h, raw=False):
                ps = psConv.tile([128, 512], F32, name="ps_conv", tag="ps_conv")
                for j in range(KC):
                    nc.tensor.matmul(
                        ps[:, :],
                        diag[br][:, (dtc * KC + j) * 128 : (dtc * KC + j + 1) * 128],
                        xp[br][dtc][:, h * HF + j : h * HF + j + HF],
                        start=(j == 0),
                        stop=(j == KC - 1),
                    )
                dst = xh[dtc] if br == "x" else zh[dtc]
                if raw:
                    # table-free DVE evac; silu applied in place later so the
                    # ACT stream isn't thrashed between Exp and Silu tables
                    nc.vector.tensor_copy(dst[:, h * HF : (h + 1) * HF], ps[:, :])
                else:
                    nc.scalar.activation(
                        out=dst[:, h * HF : (h + 1) * HF],
                        in_=ps[:, :],
                        func=AF.Silu,
                        bias=0.0,
                        scale=1.0,
                    )

            def silu_island(specs):
                for br, dtc, h in specs:
                    dst = xh[dtc] if br == "x" else zh[dtc]
                    nc.scalar.activation(
                        out=dst[:, h * HF : (h + 1) * HF],
                        in_=dst[:, h * HF : (h + 1) * HF],
                        func=AF.Silu,
                        bias=0.0,
                        scale=1.0,
                    )

            def xproj_half(h):
                RW = R + 2 * NS
                ps = psIn.tile([128, 512], F32, name="ps_xd", tag="ps_inp")
                for dtc in range(D2T):
                    nc.tensor.matmul(
                        ps[0:RW, :],
                        x_projT[:, dtc * RW : (dtc + 1) * RW],
                        xh[dtc][:, h * HF : (h + 1) * HF],
                        start=(dtc == 0),
                        stop=(dtc == D2T - 1),
                    )
                nc.vector.tensor_copy(xdbl_dt[:, h * HF : (h + 1) * HF], ps[0:R, :])
                nc.vector.tensor_copy(
                    bc_sb[:, h * HF : (h + 1) * HF], ps[R : R + 2 * NS, :]
                )
                for dst_t, rep_t in ((bbc, rep_b), (cbc, rep_c)):
                    ps2 = psIn.tile([128, 512], F32, name="ps_bc", tag="ps_inp")
                    nc.tensor.matmul(
                        ps2[:, :],
                        rep_t[:, :],
                        bc_sb[:, h * HF : (h + 1) * HF],
                        start=True,
                        stop=True,
                    )
                    nc.vector.tensor_copy(dst_t[:, h * HF : (h + 1) * HF], ps2[:, :])

            def dt_soft_half(h, du_eng):
                # Exps batched before Lns (same ACT table set covers both)
                t_sps = []
                for dtc in range(D2T):
                    ps3 = psConv.tile([128, 512], F32, name="ps_dt", tag="ps_conv")
                    nc.tensor.matmul(
                        ps3[:, :],
                        dt_projT[:, dtc * 128 : (dtc + 1) * 128],
                        xdbl_dt[:, h * HF : (h + 1) * HF],
                        start=True,
                        stop=True,
                    )
                    t_sp = tsp.tile(
                        [128, 512], F32, name=f"tsp{dtc}", tag=f"tsp{dtc}", bufs=1
                    )
                    nc.scalar.activation(
                        out=t_sp[:, :],
                        in_=ps3[:, :],
                        func=AF.Exp,
                        bias=dt_bias[:, dtc : dtc + 1],
                        scale=1.0,
                    )
                    t_sps.append(t_sp)
                for dtc in range(D2T):
                    nc.scalar.activation(
                        out=dud[dtc][:, h * 2 * HF + HF : (h + 1) * 2 * HF],
                        in_=t_sps[dtc][:, :],
                        func=AF.Ln,
                        bias=1.0,
                        scale=1.0,
                    )
                    du_eng.tensor_tensor(
                        out=dud[dtc][:, h * 2 * HF : h * 2 * HF + HF],
                        in0=dud[dtc][:, h * 2 * HF + HF : (h + 1) * 2 * HF],
                        in1=xh[dtc][:, h * HF : (h + 1) * HF],
                        op=ALU.mult,
                    )

            # ---- P1a: everything the h0 scan needs ----
            for et in range(D2T):
                in_proj_half(et, 0)
            for dtc in range(D2T):
                conv_half("x", dtc, 0)
            xproj_half(0)
            dt_soft_half(0, nc.gpsimd)

            # ---- window 1: h0 scan blocks with P1b work in the pair slots ----
            from collections import deque

            # silu-bearing convs are merged into single items so their ACT
            # table loads happen once per island, not once per scan pair
            f1 = deque()
            for et in range(D2T):
                f1.append(lambda et=et: in_proj_half(et, 1))
            f1.append(lambda: [conv_half("x", d, 1, raw=True) for d in range(D2T)])
            f1.append(lambda: silu_island([("x", d, 1) for d in range(D2T)]))
            f1.append(lambda: xproj_half(1))
            f1.append(lambda: dt_soft_half(1, nc.gpsimd))
            for et in range(D2T, ET):
                f1.append(
                    lambda et=et: (in_proj_half(et, 0), in_proj_half(et, 1))
                )
            f1.append(
                lambda: [
                    conv_half("z", d, hh, raw=True)
                    for d in range(D2T)
                    for hh in range(2)
                ]
            )
            f1.append(
                lambda: silu_island(
                    [("z", d, hh) for d in range(D2T) for hh in range(2)]
                )
            )
            w1c = DC * H // 4
            for dt in range(D2T):
                scan_block(dt, 0, f1)
                nc.scalar.dma_start(
                    out=fc1T[:, dt * w1c : (dt + 1) * w1c],
                    in_=din["fc1T"].ap()[:, dt * w1c : (dt + 1) * w1c],
                )
            while f1:
                f1.popleft()()

        # ================= window 2 + phase 3 ==========
        with (
            tc.tile_pool(name="p3", bufs=1) as p3,
            tc.tile_pool(name="t3", bufs=2) as t3,
            tc.tile_pool(name="psG3", bufs=2, space="PSUM") as psG3,
            tc.tile_pool(name="psF1", bufs=2, space="PSUM") as psF1,
            tc.tile_pool(name="psTr3", bufs=1, space="PSUM") as psTr3,
        ):
            h_res = [
                p3.tile([128, D], F32, name=f"hres{i}", tag=f"hres{i}") for i in range(TT)
            ]
            xhat2 = [
                p3.tile([128, D], BF16, name=f"xh2{i}", tag=f"xh2{i}") for i in range(TT)
            ]
            xhat2T = [
                p3.tile([128, L], BF16, name=f"xh2T{i}", tag=f"xh2T{i}")
                for i in range(DC)
            ]
            # aT holds one L-half at a time: h0 is consumed by fc2(tt 0..3)
            # before fc1_half(1) overwrites it
            aT = [
                p3.tile([128, 512], BF16, name=f"aT{i}", tag=f"aT{i}")
                for i in range(HT)
            ]

            def out_proj_tt(tt):
                ps = psG3.tile([128, D], F32, name="ps_op", tag="g3")
                korder = list(range(D2T, ET)) + list(range(D2T))
                for ki, k in enumerate(korder):
                    lhs = (
                        y_cm[k][:, tt * 128 : (tt + 1) * 128]
                        if k < D2T
                        else zh[k - D2T][:, tt * 128 : (tt + 1) * 128]
                    )
                    nc.tensor.matmul(
                        ps[:, :],
                        lhs,
                        out_projT[:, k * D : (k + 1) * D],
                        start=(ki == 0),
                        stop=(ki == ET - 1),
                    )
                # + residual on DVE (also evacuates the psum)
                nc.vector.tensor_tensor(
                    out=h_res[tt][:, :], in0=ps[:, :], in1=x_res[tt][:, :], op=ALU.add
                )

            def ln2_stats_tt(tt):
                # ACT-side stats: Square+accum and Identity+accum are in every
                # ACT table set, so they don't thrash the Exp table mid-scan
                sq = t3.tile([128, D], F32, name="sq3", tag="sq3")
                qsum = t3.tile([128, 1], F32, name="qsum", tag="qsum", bufs=8)
                ssum = t3.tile([128, 1], F32, name="ssum", tag="ssum", bufs=8)
                nc.scalar.activation(
                    out=sq[:, :], in_=h_res[tt][:, :], func=AF.Square, accum_out=qsum[:, :]
                )
                nc.scalar.activation(
                    out=sq[:, :], in_=h_res[tt][:, :], func=AF.Identity,
                    accum_out=ssum[:, :],
                )
                return qsum, ssum

            def ln2_finish_batch(tts):
                parts = {}
                for tt in tts:
                    qsum, ssum = ln2_acc[tt]
                    m = t3.tile([128, 1], F32, name="m3", tag="m3", bufs=8)
                    nc.vector.tensor_scalar(
                        out=m[:, :], in0=ssum[:, :], scalar1=1.0 / D, scalar2=None,
                        op0=ALU.mult,
                    )
                    msq = t3.tile([128, 1], F32, name="msq", tag="msq", bufs=8)
                    nc.vector.tensor_tensor(
                        out=msq[:, :], in0=m[:, :], in1=m[:, :], op=ALU.mult
                    )
                    v = t3.tile([128, 1], F32, name="v3", tag="v3", bufs=8)
                    nc.vector.scalar_tensor_tensor(
                        out=v[:, :], in0=qsum[:, :], scalar=1.0 / D, in1=msq[:, :],
                        op0=ALU.mult, op1=ALU.subtract,
                    )
                    parts[tt] = (m, v)
                sds = {}
                for tt in tts:
                    m, v = parts[tt]
                    sd = t3.tile([128, 1], F32, name="sd3", tag="sd3", bufs=8)
                    nc.scalar.activation(
                        out=sd[:, :], in_=v[:, :], func=AF.Sqrt, bias=eps_t[:, :],
                        scale=1.0,
                    )
                    sds[tt] = sd
                for tt in tts:
                    m, v = parts[tt]
                    r_t = t3.tile([128, 1], F32, name="r3", tag="r3", bufs=8)
                    nc.vector.reciprocal(out=r_t[:, :], in_=sds[tt][:, :])
                    nc.vector.tensor_scalar(
                        out=xhat2[tt][:, :],
                        in0=h_res[tt][:, :],
                        scalar1=m[:, 0:1],
                        scalar2=r_t[:, :],
                        op0=ALU.subtract,
                        op1=ALU.mult,
                    )

            def ln2_finish_tt(tt, qsum, ssum):
                # m = ssum/512; v = qsum/512 - m^2; r = 1/sqrt(v+eps); all tiny
                m = t3.tile([128, 1], F32, name="m3", tag="m3")
                nc.vector.tensor_scalar(
                    out=m[:, :], in0=ssum[:, :], scalar1=1.0 / D, scalar2=None,
                    op0=ALU.mult,
                )
                # v = qsum/512 - m*m
                msq = t3.tile([128, 1], F32, name="msq", tag="msq")
                nc.vector.tensor_tensor(
                    out=msq[:, :], in0=m[:, :], in1=m[:, :], op=ALU.mult
                )
                v = t3.tile([128, 1], F32, name="v3", tag="v3")
                nc.vector.scalar_tensor_tensor(
                    out=v[:, :], in0=qsum[:, :], scalar=1.0 / D, in1=msq[:, :],
                    op0=ALU.mult, op1=ALU.subtract,
                )
                sd = t3.tile([128, 1], F32, name="sd3", tag="sd3")
                nc.scalar.activation(
                    out=sd[:, :], in_=v[:, :], func=AF.Sqrt, bias=eps_t[:, :], scale=1.0
                )
                r_t = t3.tile([128, 1], F32, name="r3", tag="r3")
                nc.vector.reciprocal(out=r_t[:, :], in_=sd[:, :])
                nc.vector.tensor_scalar(
                    out=xhat2[tt][:, :],
                    in0=h_res[tt][:, :],
                    scalar1=m[:, 0:1],
                    scalar2=r_t[:, :],
                    op0=ALU.subtract,
                    op1=ALU.mult,
                )

            def tr3_dc(half, dc):
                ps_t = psTr3.tile([128, 512], BF16, name="ps_t3", tag="ps_t3")
                for b4 in range(4):
                    tt = half * 4 + b4
                    nc.tensor.transpose(
                        ps_t[:, b4 * 128 : (b4 + 1) * 128],
                        xhat2[tt][:, dc * 128 : (dc + 1) * 128],
                        ident_bf[:, :],
                    )
                nc.scalar.copy(
                    out=xhat2T[dc][:, half * 512 : (half + 1) * 512], in_=ps_t[:, :]
                )

            def fc1_block(half, ht0, ht1, raw=False):
                for ht in range(ht0, ht1):
                    ps = psF1.tile([128, 512], F32, name="ps_fc1", tag="f1")
                    for dc in range(DC):
                        nc.tensor.matmul(
                            ps[:, :],
                            fc1T[:, dc * H + ht * 128 : dc * H + (ht + 1) * 128],
                            xhat2T[dc][:, half * 512 : (half + 1) * 512],
                            start=(dc == 0),
                            stop=(dc == DC - 1),
                        )
                    if raw:
                        nc.vector.tensor_copy(aT[ht][:, :], ps[:, :])
                    else:
                        nc.scalar.activation(
                            out=aT[ht][:, :],
                            in_=ps[:, :],
                            func=AF.Gelu,
                            bias=c_fc1[:, ht : ht + 1],
                            scale=1.0,
                        )

            def gelu_island():
                for ht in range(HT):
                    nc.scalar.activation(
                        out=aT[ht][:, :],
                        in_=aT[ht][:, :],
                        func=AF.Gelu,
                        bias=c_fc1[:, ht : ht + 1],
                        scale=1.0,
                    )

            def fc2_tt(tt):
                ps = psG3.tile([128, D], F32, name="ps_fc2", tag="g3")
                for ht in range(HT):
                    nc.tensor.matmul(
                        ps[:, :],
                        aT[ht][:, (tt % 4) * 128 : (tt % 4 + 1) * 128],
                        fc2T[:, ht * D : (ht + 1) * D],
                        start=(ht == 0),
                        stop=False,
                    )
                nc.tensor.matmul(
                    ps[:, :], ones1[:, :], fc2b[:, :], start=False, stop=True
                )
                o_t = t3.tile([128, D], F32, name="o_t", tag="o_t")
                nc.vector.tensor_tensor(
                    out=o_t[:, :], in0=ps[:, :], in1=h_res[tt][:, :], op=ALU.add
                )
                nc.sync.dma_start(
                    out=out_ap[tt * 128 : (tt + 1) * 128, :], in_=o_t[:, :]
                )

            # ---- window 2: h1 scan blocks with phase-3 h0 in the pair slots ----
            from collections import deque

            ln2_acc = {}

            def op_item(tt):
                out_proj_tt(tt)
                ln2_acc[tt] = ln2_stats_tt(tt)

            def ln2_batch(tts):
                ln2_finish_batch(list(tts))

            f2 = deque()
            for tt in range(4):
                f2.append(lambda t=tt: op_item(t))
            f2.append(lambda: ln2_batch(range(4)))
            for dc in range(DC):
                f2.append(lambda d=dc: tr3_dc(0, d))
            # fc1 evacs are raw copies; one in-place gelu island bounds the
            # ACT table switches to a single Exp<->Gelu round trip
            f2.append(lambda: fc1_block(0, 0, 8, raw=True))
            f2.append(lambda: fc1_block(0, 8, 16, raw=True))
            f2.append(gelu_island)
            for tt in range(4):
                f2.append(lambda t=tt: fc2_tt(t))

            # fc2T must be emitted before the fc2 filler items (tile deps
            # bind at emission order); stream in two chunks up front
            w2c = HT * D // 2
            for ck in range(2):
                nc.scalar.dma_start(
                    out=fc2T[:, ck * w2c : (ck + 1) * w2c],
                    in_=din["fc2T"].ap()[:, ck * w2c : (ck + 1) * w2c],
                )
            for dt in range(D2T):
                scan_block(dt, 1, f2)

            # ---- tail: phase-3 for h1 ----
            while f2:
                f2.popleft()()
            for tt in range(4, TT):
                op_item(tt)
            ln2_batch(range(4, TT))
            for dc in range(DC):
                tr3_dc(1, dc)
            fc1_block(1, 0, 16)
            for tt in range(4, TT):
                fc2_tt(tt)


def _mk_repbc(row0):
    m = np.zeros((2 * NS, 128), np.float32)
    p = np.arange(128)
    m[row0 + (p % 16), p] = 1.0
    return m


def prep_inputs(inputs):
    """Host-side weight preprocessing. Returns the shared (non-x) in_map."""
    g = {k: np.asarray(v, dtype=np.float32) for k, v in inputs.items()}

    ln1_w, ln1_b = g["ln1_w"], g["ln1_b"]
    ln2_w, ln2_b = g["ln2_w"], g["ln2_b"]

    w_in = g["in_proj_w"] * ln1_w[None, :]  # [E, D]
    c_in = (g["in_proj_w"] @ ln1_b).astype(np.float32)  # [E]

    fc1 = g["fc1_w"] * ln2_w[None, :]  # [H, D]
    c_fc1 = (g["fc1_w"] @ ln2_b + g["fc1_b"]).astype(np.float32)  # [H]

    A = -np.exp(g["A_log"])  # [D2, NS]

    # REP[q][k, m] = 1 iff k == q*8 + m//16   (delta row broadcast)
    rep = np.zeros((16, 128, 128), np.float32)
    for q in range(16):
        m = np.arange(128)
        rep[q, q * 8 + m // 16, m] = 1.0
    # SEL[q][k, m] = 1 iff m == q*8 + k//16   (sum over n into channel rows)
    sel = np.transpose(rep, (0, 2, 1)).copy()
    # A_perm[p, g] = A[g*8 + p//16, p%16]
    p = np.arange(128)
    gg = np.arange(NG)
    A_perm = A[(gg[None, :] * 8 + (p // 16)[:, None]), (p % 16)[:, None]].astype(
        np.float32
    )
    # shared-scale fast path: A rows identical across channels (A[c,n] = v[n])
    v = A[0, :]
    a_shared = bool(np.allclose(A, v[None, :], rtol=1e-5, atol=1e-7))
    v_col = v[(p % 16)].reshape(128, 1).astype(np.float32)
    diag_D = np.zeros((D2T, 128, 128), np.float32)
    idx128 = np.arange(128)
    for dt in range(D2T):
        diag_D[dt, idx128, idx128] = g["ssm_D"][dt * 128 : (dt + 1) * 128]

    conv_x = g["conv_x_w"][:, 0, :]  # [D2, KC]
    conv_z = g["conv_z_w"][:, 0, :]
    diag_x = np.zeros((D2T * KC, 128, 128), np.float32)
    diag_z = np.zeros((D2T * KC, 128, 128), np.float32)
    idx = np.arange(128)
    for dt in range(D2T):
        for j in range(KC):
            diag_x[dt * KC + j, idx, idx] = conv_x[dt * 128 : (dt + 1) * 128, j]
            diag_z[dt * KC + j, idx, idx] = conv_z[dt * 128 : (dt + 1) * 128, j]

    def bf(x):
        return np.ascontiguousarray(x.astype(_BF))

    def f(x):
        return np.ascontiguousarray(x.astype(np.float32))

    def blocks(arr3):  # [N, 128, W] -> [128, N*W]
        n, pdim, w = arr3.shape
        assert pdim == 128
        return np.ascontiguousarray(arr3.transpose(1, 0, 2).reshape(128, n * w))

    RW = R + 2 * NS
    shared = {
        "w_inT": bf(blocks(w_in.T.reshape(DC, 128, E))),
        "c_in": f(c_in.reshape(ET, 128).T),
        "diag_x": bf(blocks(diag_x)),
        "diag_z": bf(blocks(diag_z)),
        "x_projT": bf(blocks(g["x_proj_w"].T.reshape(D2T, 128, RW))),
        "dt_projT": bf(g["dt_proj_w"].T),
        "dt_bias": f(g["dt_proj_b"].reshape(D2T, 128).T),
        "A_perm": f(A_perm),
        "v_col": f(v_col),
        "diag_D": bf(blocks(diag_D)),
        "sel": bf(blocks(sel)),
        "out_projT": bf(blocks(g["out_proj_w"].T.reshape(ET, 128, D))),
        "fc1T": bf(blocks(fc1.T.reshape(DC, 128, H))),
        "c_fc1": f(c_fc1.reshape(HT, 128).T),
        "fc2T": bf(blocks(g["fc2_w"].T.reshape(HT, 128, D))),
        "fc2b": f(g["fc2_b"].reshape(1, D)),
        "ident_bf": bf(np.eye(128, dtype=np.float32)),
        "zpad": np.zeros((128, 3), _BF),
        "ones1d": np.ones((1, 128), np.float32),
        "rep_b": bf(_mk_repbc(0)),
        "rep_c": bf(_mk_repbc(NS)),
        "ident_f": np.eye(128, dtype=np.float32),
    }
    return shared, a_shared


_CACHED_NC = {}


def kernel(**inputs):
    from concourse.bass_utils import run_bass_kernel_spmd

    shared, a_shared = prep_inputs(inputs)
    if a_shared not in _CACHED_NC:
        _CACHED_NC[a_shared] = build_kernel(shared_scale=a_shared)
    nc = _CACHED_NC[a_shared]
    x = np.asarray(inputs["x"], dtype=np.float32)
    in_maps = [
        dict(
            shared,
            xin=np.ascontiguousarray(
                np.concatenate([x[i]] * KREPEAT, axis=0).astype(_BF)
            ),
        )
        for i in range(NCORES)
    ]
    res = run_bass_kernel_spmd(nc, in_maps, core_ids=list(range(NCORES)))
    out = np.stack([res.results[i]["out"][:L] for i in range(NCORES)], axis=0)
    return out


if __name__ == "__main__":
    nc = build_kernel()
    print("build ok")

